# revision 13
# baseline (speedup 1.0000x reference)
"""Chunked local attention (landmark-augmented) for 8 Trainium2 NeuronCores.

Model (see reference): B=4, S=8192, D=768, H=12 heads of 64, chunk=512,
NL=32 landmark tokens = mean of 32 evenly spaced 256-token segments.
Every chunk attends over [32 landmarks ; its own 512 tokens].

Sharding: core c handles batch b=c//2, tokens [h*4096, (h+1)*4096), h=c%2.
Each core computes the 16 landmark partial means of its own half-sequence
and all-gathers the other 16 from its pair core ({2b, 2b+1}).

Layout strategy (all matmuls in fp32r, 1 cycle/row at N>=256):
  - host passes x shard both natural [4096,768] (landmark sums only) and
    transposed [768,4096] (everything else), plus W.T weights [din,dout].
  - Q_T, K_T computed transposed [768, 512] per chunk; V natural [512, 768].
  - scores computed transposed [ctx, 512] so softmax-sum and attn@V both
    contract over ctx on the partition dim; Z row-sums via ones-matmuls.
  - attention out O_T [768, 512] feeds the output projection directly.
"""

import sys

for _p in ("/opt/trn_rl_repo", "/root/.axon_site/_ro/trn_rl_repo"):
    if _p not in sys.path:
        sys.path.append(_p)

import numpy as np

import concourse.bacc as bacc
import concourse.tile as tile
from concourse import mybir
from concourse import bass_utils

B, S, D = 4, 8192, 768
H, HD = 12, 64
CS, NL = 512, 32
N_CORES = 8
TOK = B * S // N_CORES      # 4096 tokens per core
NCH = TOK // CS             # 8 chunks per core
SEG = S // NL               # 256 tokens per landmark segment
NSEG_LOC = TOK // SEG       # 16 local segments per core
DT = D // 128               # 6 din/dout tiles
P = 128
SCALE = HD ** -0.5          # 0.125

F32 = mybir.dt.float32
F32R = mybir.dt.float32r
BF16 = mybir.dt.bfloat16
EXP = mybir.ActivationFunctionType.Exp
IDENT = mybir.ActivationFunctionType.Identity

_CACHE = {}


def _build_nc(debug_dump=False):
    nc = bacc.Bacc("TRN2", target_bir_lowering=False, debug=False,
                   num_devices=N_CORES)

    # ---- DRAM I/O --------------------------------------------------------
    xt_d = nc.dram_tensor("xt", [D, TOK], F32, kind="ExternalInput")
    xn_d = nc.dram_tensor("xn", [TOK, D], F32, kind="ExternalInput")
    w_d = {k: nc.dram_tensor(f"w{k}", [D, D], F32, kind="ExternalInput")
           for k in "qkvo"}
    qb_d = nc.dram_tensor("qb", [P, DT], F32, kind="ExternalInput")
    kb_d = nc.dram_tensor("kb", [P, DT], F32, kind="ExternalInput")
    vb_d = nc.dram_tensor("vb", [1, D], F32, kind="ExternalInput")
    ob_d = nc.dram_tensor("ob", [1, D], F32, kind="ExternalInput")
    y_d = nc.dram_tensor("y", [TOK, D], F32, kind="ExternalOutput")
    dbg = {}
    if debug_dump:
        for nm, shp in [("lm_loc", [NSEG_LOC, D]), ("lm_nat", [NL, D]),
                        ("lm_T0", [P, NL]), ("klm_T0", [P, NL]),
                        ("vlm_d", [NL, D]), ("qT00", [P, CS]),
                        ("kT00", [P, CS]), ("v00", [P, D]),
                        ("e0_p0k0", [NL, CS]), ("e0_p0k1", [P, CS]),
                        ("e1_p0k1", [P, CS]),
                        ("z_p0", [P, CS]), ("rzb_p0", [P, CS]),
                        ("oT00", [P, CS])]:
            dbg[nm] = nc.dram_tensor("dbg_" + nm, shp, F32, kind="ExternalOutput")

    def dump(nm, ap):
        if debug_dump and nm in dbg:
            t = sb.tile(list(ap.shape), F32, name="dmp_" + nm, tag="dmp", bufs=2)
            nc.vector.tensor_copy(t[:], ap)
            nc.sync.dma_start(dbg[nm][tuple(slice(0, s) for s in ap.shape)], t[:])

    with tile.TileContext(nc) as tc:
        with (
            tc.tile_pool(name="wpool", bufs=1) as wpool,
            tc.tile_pool(name="singles", bufs=1) as singles,
            tc.tile_pool(name="sb", bufs=2) as sb,
            tc.tile_pool(name="psa", bufs=2, space="PSUM") as psa,
            tc.tile_pool(name="pss", bufs=3, space="PSUM") as pss,
            tc.tile_pool(name="pso", bufs=2, space="PSUM") as pso_pool,
            tc.tile_pool(name="psz", bufs=1, space="PSUM") as psz_pool,
            tc.tile_pool(name="dram", bufs=1, space="DRAM") as dram,
        ):
            # ---- constants / weights ------------------------------------
            ones = singles.tile([P, 1], BF16, name="ones")
            nc.vector.memset(ones[:], 1.0)
            # indicator patterns for landmark sums: ind[:, 16*s + s] = 1
            ind_f = singles.tile([P, 16 * NSEG_LOC], F32, name="ind_f")
            nc.vector.memset(ind_f[:], 0.0)
            for s in range(NSEG_LOC):
                nc.vector.memset(ind_f[:, 16 * s + s: 16 * s + s + 1], 1.0)
            ind = singles.tile([P, 16 * NSEG_LOC], F32R, name="ind")
            nc.scalar.activation(ind[:], ind_f[:], IDENT)
            ident32 = singles.tile([32, 32], F32, name="ident32")
            from concourse.masks import make_identity
            make_identity(nc, ident32[:])

            wsb = {}
            for k in "qkvo":
                for d in range(DT):
                    t = wpool.tile([P, D], F32R, name=f"w{k}{d}", tag=f"w{k}{d}")
                    nc.sync.dma_start(t[:], w_d[k][d * P:(d + 1) * P, :].bitcast(F32R))
                    wsb[k, d] = t
            qb_sb = singles.tile([P, DT], F32, name="qb_sb")
            kb_sb = singles.tile([P, DT], F32, name="kb_sb")
            nc.sync.dma_start(qb_sb[:], qb_d[:, :])
            nc.sync.dma_start(kb_sb[:], kb_d[:, :])
            vb_bc = singles.tile([P, D], F32, name="vb_bc")
            ob_bc = singles.tile([P, D], F32, name="ob_bc")
            nc.sync.dma_start(vb_bc[:], vb_d[0:1, :].partition_broadcast(P))
            nc.sync.dma_start(ob_bc[:], ob_d[0:1, :].partition_broadcast(P))

            # ---- landmark partial sums (natural layout, via indicators) --
            # landmark accumulators borrow the attention-output psum slots:
            # those are first needed by chunk-0 attention, which already
            # depends on the landmarks, so this adds no serialization.
            lm_ps = pso_pool.tile([NSEG_LOC, 512], F32, name="lm_ps", tag="pso")
            lm_ps2 = pso_pool.tile([NSEG_LOC, 256], F32, name="lm_ps2", tag="pso")
            for tt in range(TOK // P):  # 32 token tiles
                xn_t = sb.tile([P, D], F32R, name=f"xn{tt}", tag="v", bufs=5)
                nc.sync.dma_start(
                    xn_t[:], xn_d[tt * P:(tt + 1) * P, :].bitcast(F32R))
                s = tt * P // SEG
                lhs = ind[:, 16 * s: 16 * (s + 1)]
                nc.tensor.matmul(lm_ps[:, :], lhs, xn_t[:, 0:512],
                                 start=(tt == 0), stop=(tt == TOK // P - 1))
                nc.tensor.matmul(lm_ps2[:, :], lhs, xn_t[:, 512:D],
                                 start=(tt == 0), stop=(tt == TOK // P - 1))

            lm_loc = sb.tile([NSEG_LOC, D], F32, name="lm_loc", tag="rz", bufs=2)
            nc.scalar.copy(lm_loc[:, 0:512], lm_ps[:, :])
            nc.scalar.copy(lm_loc[:, 512:D], lm_ps2[:, :])
            dump("lm_loc", lm_loc[:, :])
            lm_in_b = dram.tile([NSEG_LOC, D], F32, name="lm_in_b")
            lm_out_b = dram.tile([NL, D], F32, name="lm_out_b")
            nc.sync.dma_start(lm_in_b[:, :], lm_loc[:, :])
            nc.gpsimd.collective_compute(
                "AllGather", mybir.AluOpType.bypass,
                replica_groups=[[0, 1], [2, 3], [4, 5], [6, 7]],
                ins=[lm_in_b.opt()], outs=[lm_out_b.opt()],
            )
            lm_nat = sb.tile([NL, D], F32, name="lm_nat", tag="v", bufs=5)
            nc.sync.dma_start(lm_nat[:], lm_out_b[:, :])

            dump("lm_nat", lm_nat[:, :])
            # transpose to lm_T [din, 32] per din tile, scaled by 1/SEG
            lm_T = []
            for d in range(DT):
                ps_t = pss.tile([P, NL], F32, name=f"lmT_ps{d}", tag="pss")
                nc.tensor.transpose(ps_t[:], lm_nat[:, d * P:(d + 1) * P],
                                    ident32[:])
                t = singles.tile([P, NL], F32R, name=f"lmT{d}")
                nc.scalar.activation(t[:], ps_t[:], IDENT, scale=1.0 / SEG)
                if d == 0:
                    dump("lm_T0", t[:])
                lm_T.append(t)

            # K_lm_T [dout, 32] per dout tile (+ k bias); V_lm [32, 768] (+ v bias)
            klm_T = []
            for m in range(DT):
                ps_k = pss.tile([P, NL], F32, name=f"klm_ps{m}", tag="pss")
                for d in range(DT):
                    nc.tensor.matmul(ps_k[:], wsb["k", d][:, m * P:(m + 1) * P],
                                     lm_T[d][:], start=(d == 0), stop=(d == DT - 1))
                t = singles.tile([P, NL], F32R, name=f"klmT{m}")
                nc.scalar.activation(t[:], ps_k[:], IDENT, bias=kb_sb[:, m:m + 1])
                if m == 0:
                    dump("klm_T0", t[:])
                klm_T.append(t)
            ps_v1 = pss.tile([NL, 512], F32, name="ps_vlm1", tag="pss")
            ps_v2 = pss.tile([NL, 256], F32, name="ps_vlm2", tag="pss")
            for d in range(DT):
                nc.tensor.matmul(ps_v1[:], lm_T[d][:], wsb["v", d][:, 0:512],
                                 start=(d == 0), stop=(d == DT - 1))
                nc.tensor.matmul(ps_v2[:], lm_T[d][:], wsb["v", d][:, 512:D],
                                 start=(d == 0), stop=(d == DT - 1))
            vlm = singles.tile([NL, D], BF16, name="vlm")
            nc.vector.tensor_add(vlm[:, 0:512], ps_v1[:], vb_bc[0:NL, 0:512])
            nc.vector.tensor_add(vlm[:, 512:D], ps_v2[:], vb_bc[0:NL, 512:D])

            dump("vlm_d", vlm[:, :])
            # ---- main chunk pipeline ------------------------------------
            for c in range(NCH):
                T0 = c * CS
                xt_c = []
                for d in range(DT):
                    t = sb.tile([P, CS], F32R, name=f"xt{c}_{d}", tag="xt", bufs=9)
                    nc.sync.dma_start(
                        t[:], xt_d[d * P:(d + 1) * P, T0:T0 + CS].bitcast(F32R))
                    xt_c.append(t)

                # Q_T (scaled by SCALE, bias pre-scaled on host), K_T
                qT, kT = [], []
                for m in range(DT):
                    ps_q = psa.tile([P, CS], F32, name=f"q_ps{c}_{m}", tag="psa")
                    for d in range(DT):
                        nc.tensor.matmul(ps_q[:], wsb["q", d][:, m * P:(m + 1) * P],
                                         xt_c[d][:], start=(d == 0),
                                         stop=(d == DT - 1))
                    t = sb.tile([P, CS], F32R, name=f"qT{c}_{m}", tag="qT", bufs=8)
                    nc.scalar.activation(t[:], ps_q[:], IDENT, scale=SCALE,
                                         bias=qb_sb[:, m:m + 1])
                    if c == 0 and m == 0:
                        dump("qT00", t[:])
                    qT.append(t)
                for m in range(DT):
                    ps_k = psa.tile([P, CS], F32, name=f"k_ps{c}_{m}", tag="psa")
                    for d in range(DT):
                        nc.tensor.matmul(ps_k[:], wsb["k", d][:, m * P:(m + 1) * P],
                                         xt_c[d][:], start=(d == 0),
                                         stop=(d == DT - 1))
                    t = sb.tile([P, CS], F32R, name=f"kT{c}_{m}", tag="kT", bufs=8)
                    nc.scalar.activation(t[:], ps_k[:], IDENT,
                                         bias=kb_sb[:, m:m + 1])
                    if c == 0 and m == 0:
                        dump("kT00", t[:])
                    kT.append(t)

                # V natural [512, 768] as 4 tiles of [128, 768]
                vt = []
                for tt in range(CS // P):
                    ps_1 = psa.tile([P, 512], F32, name=f"v1_ps{c}_{tt}", tag="psa")
                    ps_2 = psa.tile([P, 256], F32, name=f"v2_ps{c}_{tt}", tag="psa")
                    for d in range(DT):
                        nc.tensor.matmul(ps_1[:], xt_c[d][:, tt * P:(tt + 1) * P],
                                         wsb["v", d][:, 0:512], start=(d == 0),
                                         stop=(d == DT - 1))
                        nc.tensor.matmul(ps_2[:], xt_c[d][:, tt * P:(tt + 1) * P],
                                         wsb["v", d][:, 512:D], start=(d == 0),
                                         stop=(d == DT - 1))
                    t = sb.tile([P, D], BF16, name=f"v{c}_{tt}", tag="v", bufs=5)
                    nc.vector.tensor_add(t[:, 0:512], ps_1[:], vb_bc[:, 0:512])
                    nc.vector.tensor_add(t[:, 512:D], ps_2[:], vb_bc[:, 512:D])
                    if c == 0 and tt == 0:
                        dump("v00", t[:])
                    vt.append(t)

                # attention, head pairs p: heads 2p (rows 0:64), 2p+1 (64:128)
                oT = []
                for p in range(DT):
                    ps_o = pso_pool.tile([P, CS], F32, name=f"o_ps{c}_{p}",
                                         tag="pso")
                    ps_z = psz_pool.tile([P, CS], F32, name=f"z_ps{c}_{p}",
                                         tag="psz")
                    for kt in range(5):
                        if kt == 0:
                            ksz = NL
                            k0 = klm_T[p][0:64, :]
                            k1 = klm_T[p][64:P, :]
                            v0 = vlm[0:NL, p * P: p * P + 64]
                            v1 = vlm[0:NL, p * P + 64: (p + 1) * P]
                        else:
                            j = kt - 1
                            ksz = P
                            k0 = kT[p][0:64, j * P:(j + 1) * P]
                            k1 = kT[p][64:P, j * P:(j + 1) * P]
                            v0 = vt[j][:, p * P: p * P + 64]
                            v1 = vt[j][:, p * P + 64: (p + 1) * P]
                        s0 = pss.tile([P, CS], F32, name=f"s0_{c}_{p}_{kt}",
                                      tag="pss")
                        s1 = pss.tile([P, CS], F32, name=f"s1_{c}_{p}_{kt}",
                                      tag="pss")
                        nc.tensor.matmul(s0[0:ksz, :], k0, qT[p][0:64, :],
                                         start=True, stop=True)
                        nc.tensor.matmul(s1[0:ksz, :], k1, qT[p][64:P, :],
                                         start=True, stop=True)
                        e0 = sb.tile([P, CS], BF16, name=f"e0_{c}_{p}_{kt}",
                                     tag="e", bufs=6)
                        e1 = sb.tile([P, CS], BF16, name=f"e1_{c}_{p}_{kt}",
                                     tag="e", bufs=6)
                        nc.scalar.activation(e0[0:ksz, :], s0[0:ksz, :], EXP)
                        nc.scalar.activation(e1[0:ksz, :], s1[0:ksz, :], EXP)
                        if c == 0 and p == 0 and kt == 0:
                            dump("e0_p0k0", e0[0:ksz, :])
                        if c == 0 and p == 0 and kt == 1:
                            dump("e0_p0k1", e0[0:ksz, :])
                            dump("e1_p0k1", e1[0:ksz, :])
                        nc.tensor.matmul(ps_o[0:64, :], v0[0:ksz, :], e0[0:ksz, :],
                                         start=(kt == 0), stop=(kt == 4),
                                         tile_position=(0, 0))
                        nc.tensor.matmul(ps_o[64:P, :], v1[0:ksz, :], e1[0:ksz, :],
                                         start=(kt == 0), stop=(kt == 4),
                                         tile_position=(0, 64))
                        nc.tensor.matmul(ps_z[0:1, :], ones[0:ksz, :], e0[0:ksz, :],
                                         start=(kt == 0), stop=(kt == 4),
                                         tile_position=(0, 0))
                        nc.tensor.matmul(ps_z[32:33, :], ones[0:ksz, :],
                                         e1[0:ksz, :], start=(kt == 0),
                                         stop=(kt == 4), tile_position=(0, 32))
                    if c == 0 and p == 0:
                        dump("z_p0", ps_z[:, :])
                    rz = sb.tile([33, CS], F32, name=f"rz{c}_{p}", tag="rz",
                                 bufs=2)
                    nc.vector.reciprocal(rz[0:1, :], ps_z[0:1, :])
                    nc.vector.reciprocal(rz[32:33, :], ps_z[32:33, :])
                    zdram = dram.tile([2, CS], F32, name=f"zd{c}_{p}", tag="zd",
                                      bufs=4)
                    nc.sync.dma_start(zdram[0:1, :], rz[0:1, :])
                    nc.sync.dma_start(zdram[1:2, :], rz[32:33, :])
                    rzb = sb.tile([P, CS], F32, name=f"rzb{c}_{p}", tag="rzb",
                                  bufs=3)
                    nc.sync.dma_start(rzb[0:64, :],
                                      zdram[0:1, :].partition_broadcast(64))
                    nc.sync.dma_start(rzb[64:P, :],
                                      zdram[1:2, :].partition_broadcast(64))
                    if c == 0 and p == 0:
                        dump("rzb_p0", rzb[:, :])
                    t = sb.tile([P, CS], F32R, name=f"oT{c}_{p}", tag="oT", bufs=8)
                    nc.vector.tensor_mul(t[:], ps_o[:], rzb[:])
                    if c == 0 and p == 0:
                        dump("oT00", t[:])
                    oT.append(t)

                # output projection Y = O_T.T @ WoT + ob
                for tt in range(CS // P):
                    ps_1 = psa.tile([P, 512], F32, name=f"y1_ps{c}_{tt}", tag="psa")
                    ps_2 = psa.tile([P, 256], F32, name=f"y2_ps{c}_{tt}", tag="psa")
                    for d in range(DT):
                        nc.tensor.matmul(ps_1[:], oT[d][:, tt * P:(tt + 1) * P],
                                         wsb["o", d][:, 0:512], start=(d == 0),
                                         stop=(d == DT - 1))
                        nc.tensor.matmul(ps_2[:], oT[d][:, tt * P:(tt + 1) * P],
                                         wsb["o", d][:, 512:D], start=(d == 0),
                                         stop=(d == DT - 1))
                    t = sb.tile([P, D], F32, name=f"y{c}_{tt}", tag="y", bufs=3)
                    nc.vector.tensor_add(t[:, 0:512], ps_1[:], ob_bc[:, 0:512])
                    nc.vector.tensor_add(t[:, 512:D], ps_2[:], ob_bc[:, 512:D])
                    nc.sync.dma_start(
                        y_d[T0 + tt * P: T0 + (tt + 1) * P, :], t[:])

    nc.finalize()
    return nc


def _get_nc(debug_dump=False):
    key = ("nc", debug_dump)
    if key not in _CACHE:
        _CACHE[key] = _build_nc(debug_dump)
    return _CACHE[key]


def _make_in_maps(x, q_w, q_b, k_w, k_b, v_w, v_b, o_w, o_b):
    x = np.asarray(x, np.float32)
    shared = {
        "wq": np.ascontiguousarray(np.asarray(q_w, np.float32).T),
        "wk": np.ascontiguousarray(np.asarray(k_w, np.float32).T),
        "wv": np.ascontiguousarray(np.asarray(v_w, np.float32).T),
        "wo": np.ascontiguousarray(np.asarray(o_w, np.float32).T),
        "qb": np.ascontiguousarray(
            (np.asarray(q_b, np.float32) * SCALE).reshape(DT, P).T),
        "kb": np.ascontiguousarray(np.asarray(k_b, np.float32).reshape(DT, P).T),
        "vb": np.asarray(v_b, np.float32).reshape(1, D).copy(),
        "ob": np.asarray(o_b, np.float32).reshape(1, D).copy(),
    }
    in_maps = []
    for c in range(N_CORES):
        b, h = divmod(c, 2)
        xs = x[b, h * TOK:(h + 1) * TOK, :]
        m = dict(shared)
        m["xt"] = np.ascontiguousarray(xs.T)
        m["xn"] = np.ascontiguousarray(xs)
        in_maps.append(m)
    return in_maps


def run(trace=False, trace_cores=None, debug_dump=False, **inputs):
    nc = _get_nc(debug_dump)
    in_maps = _make_in_maps(**inputs)
    res = bass_utils.run_bass_kernel_spmd(
        nc, in_maps, core_ids=list(range(N_CORES)), trace=trace,
        trace_cores=trace_cores)
    out = np.empty((B, S, D), np.float32)
    for c in range(N_CORES):
        b, h = divmod(c, 2)
        out[b, h * TOK:(h + 1) * TOK, :] = res.results[c]["y"]
    return out, res


def kernel(**inputs) -> np.ndarray:
    out, _ = run(trace=False, **inputs)
    return out


# revision 16
# speedup vs baseline: 1.1321x; 1.1321x over previous
"""Chunked local attention (landmark-augmented) for 8 Trainium2 NeuronCores.

Model (see reference): B=4, S=8192, D=768, H=12 heads of 64, chunk=512,
NL=32 landmark tokens = mean of 32 evenly spaced 256-token segments.
Every chunk attends over [32 landmarks ; its own 512 tokens].

Sharding: core c handles batch b=c//2, tokens [h*4096, (h+1)*4096), h=c%2.
Each core computes the 16 landmark partial means of its own half-sequence
and all-gathers the other 16 from its pair core ({2b, 2b+1}).

Layout strategy (matmuls in bf16, fp32 psum; weight loads hide via FWL):
  - host passes x shard both natural [4096,768] (landmark sums only) and
    transposed [768,4096] (everything else), plus W.T weights [din,dout],
    all pre-cast to bf16.
  - Q_T, K_T computed transposed [768, 512] per chunk; V natural [512, 768].
  - scores computed transposed [ctx, 512] so softmax-sum and attn@V both
    contract over ctx on the partition dim; Z row-sums via ones-matmuls
    packed into free column strips of the array.
  - attention out O_T [768, 512] feeds the output projection directly.
  - emission is software-pipelined: chunk c's output projection is emitted
    after chunk c+1's QKV so the in-order PE queue never stalls on the
    softmax-normalize DMA round trip.
"""

import sys

for _p in ("/opt/trn_rl_repo", "/root/.axon_site/_ro/trn_rl_repo"):
    if _p not in sys.path:
        sys.path.append(_p)

import numpy as np
import ml_dtypes

import concourse.bacc as bacc
import concourse.tile as tile
from concourse import mybir
from concourse import bass_utils

B, S, D = 4, 8192, 768
H, HD = 12, 64
CS, NL = 512, 32
N_CORES = 8
TOK = B * S // N_CORES      # 4096 tokens per core
NCH = TOK // CS             # 8 chunks per core
SEG = S // NL               # 256 tokens per landmark segment
NSEG_LOC = TOK // SEG       # 16 local segments per core
DT = D // 128               # 6 din/dout tiles
P = 128
SCALE = HD ** -0.5          # 0.125

F32 = mybir.dt.float32
BF16 = mybir.dt.bfloat16
NPBF = ml_dtypes.bfloat16
EXP = mybir.ActivationFunctionType.Exp
IDENT = mybir.ActivationFunctionType.Identity

_CACHE = {}


def _build_nc(debug_dump=False):
    nc = bacc.Bacc("TRN2", target_bir_lowering=False, debug=False,
                   num_devices=N_CORES)

    # ---- DRAM I/O --------------------------------------------------------
    xt_d = nc.dram_tensor("xt", [D, TOK], BF16, kind="ExternalInput")
    xn_d = nc.dram_tensor("xn", [TOK, D], BF16, kind="ExternalInput")
    w_d = {k: nc.dram_tensor(f"w{k}", [D, D], BF16, kind="ExternalInput")
           for k in "qkvo"}
    qb_d = nc.dram_tensor("qb", [P, DT], F32, kind="ExternalInput")
    kb_d = nc.dram_tensor("kb", [P, DT], F32, kind="ExternalInput")
    vb_d = nc.dram_tensor("vb", [1, D], F32, kind="ExternalInput")
    ob_d = nc.dram_tensor("ob", [1, D], F32, kind="ExternalInput")
    y_d = nc.dram_tensor("y", [TOK, D], F32, kind="ExternalOutput")

    dbg = {}

    with tile.TileContext(nc) as tc:
        with (
            tc.tile_pool(name="wpool", bufs=1) as wpool,
            tc.tile_pool(name="singles", bufs=1) as singles,
            tc.tile_pool(name="sb", bufs=2) as sb,
            tc.tile_pool(name="psa", bufs=2, space="PSUM") as psa,
            tc.tile_pool(name="pss", bufs=3, space="PSUM") as pss,
            tc.tile_pool(name="pso", bufs=2, space="PSUM") as pso_pool,
            tc.tile_pool(name="psz", bufs=1, space="PSUM") as psz_pool,
            tc.tile_pool(name="dram", bufs=1, space="DRAM") as dram,
        ):
            if debug_dump:
                for nm, shp in [("lm_loc", [NSEG_LOC, D]), ("lm_nat", [NL, D]),
                                ("lm_T0", [P, NL]), ("klm_T0", [P, NL]),
                                ("vlm_d", [NL, D]), ("qT00", [P, CS]),
                                ("kT00", [P, CS]), ("v00", [P, D]),
                                ("e0_p0k0", [NL, CS]), ("e0_p0k1", [P, CS]),
                                ("e1_p0k1", [P, CS]), ("z_p0", [P, CS]),
                                ("rzb_p0", [P, CS]), ("oT00", [P, CS])]:
                    dbg[nm] = nc.dram_tensor("dbg_" + nm, shp, F32,
                                             kind="ExternalOutput")

            def dump(nm, ap):
                if nm in dbg:
                    t = sb.tile(list(ap.shape), F32, name="dmp_" + nm,
                                tag="dmp", bufs=2)
                    nc.vector.tensor_copy(t[:], ap)
                    nc.sync.dma_start(
                        dbg[nm][tuple(slice(0, s) for s in ap.shape)], t[:])

            # ---- constants / weights ------------------------------------
            ones = singles.tile([P, 1], BF16, name="ones")
            nc.vector.memset(ones[:], 1.0)
            # indicator patterns for landmark sums: ind[:, 16*s + s] = 1
            ind = singles.tile([P, 16 * NSEG_LOC], BF16, name="ind")
            nc.vector.memset(ind[:], 0.0)
            for s in range(NSEG_LOC):
                nc.vector.memset(ind[:, 16 * s + s: 16 * s + s + 1], 1.0)
            ident32 = singles.tile([32, 32], F32, name="ident32")
            from concourse.masks import make_identity
            make_identity(nc, ident32[:])

            wsb = {}
            for k in "qkvo":
                for d in range(DT):
                    t = wpool.tile([P, D], BF16, name=f"w{k}{d}", tag=f"w{k}{d}")
                    nc.scalar.dma_start(t[:], w_d[k][d * P:(d + 1) * P, :])
                    wsb[k, d] = t
            qb_sb = singles.tile([P, DT], F32, name="qb_sb")
            kb_sb = singles.tile([P, DT], F32, name="kb_sb")
            nc.scalar.dma_start(qb_sb[:], qb_d[:, :])
            nc.scalar.dma_start(kb_sb[:], kb_d[:, :])
            vb_bc = singles.tile([P, D], F32, name="vb_bc")
            ob_bc = singles.tile([P, D], F32, name="ob_bc")
            nc.scalar.dma_start(vb_bc[:], vb_d[0:1, :].partition_broadcast(P))
            nc.scalar.dma_start(ob_bc[:], ob_d[0:1, :].partition_broadcast(P))

            # ---- landmark partial sums (natural layout, via indicators) --
            # landmark accumulators borrow the attention-output psum slots:
            # those are first needed by chunk-0 attention, which already
            # depends on the landmarks, so this adds no serialization.
            lm_ps = pso_pool.tile([NSEG_LOC, 512], F32, name="lm_ps", tag="pso")
            lm_ps2 = pso_pool.tile([NSEG_LOC, 256], F32, name="lm_ps2", tag="pso")
            for tt in range(TOK // P):  # 32 token tiles
                xn_t = sb.tile([P, D], BF16, name=f"xn{tt}", tag="v", bufs=6)
                nc.gpsimd.dma_start(xn_t[:], xn_d[tt * P:(tt + 1) * P, :])
                s = tt * P // SEG
                lhs = ind[:, 16 * s: 16 * (s + 1)]
                nc.tensor.matmul(lm_ps[:, :], lhs, xn_t[:, 0:512],
                                 start=(tt == 0), stop=(tt == TOK // P - 1))
                nc.tensor.matmul(lm_ps2[:, :], lhs, xn_t[:, 512:D],
                                 start=(tt == 0), stop=(tt == TOK // P - 1))

            lm_loc = sb.tile([NSEG_LOC, D], F32, name="lm_loc", tag="rz", bufs=3)
            nc.scalar.copy(lm_loc[:, 0:512], lm_ps[:, :])
            nc.scalar.copy(lm_loc[:, 512:D], lm_ps2[:, :])
            dump("lm_loc", lm_loc[:, :])
            lm_in_b = dram.tile([NSEG_LOC, D], F32, name="lm_in_b")
            lm_out_b = dram.tile([NL, D], F32, name="lm_out_b")
            nc.gpsimd.dma_start(lm_in_b[:, :], lm_loc[:, :])
            nc.gpsimd.collective_compute(
                "AllGather", mybir.AluOpType.bypass,
                replica_groups=[[0, 1], [2, 3], [4, 5], [6, 7]],
                ins=[lm_in_b.opt()], outs=[lm_out_b.opt()],
            )
            lm_nat = sb.tile([NL, D], F32, name="lm_nat", tag="rz", bufs=3)
            nc.gpsimd.dma_start(lm_nat[:], lm_out_b[:, :])
            dump("lm_nat", lm_nat[:, :])

            # transpose to lm_T [din, 32] per din tile, scaled by 1/SEG
            lm_T = []
            for d in range(DT):
                ps_t = pss.tile([P, NL], F32, name=f"lmT_ps{d}", tag="pss")
                nc.tensor.transpose(ps_t[:], lm_nat[:, d * P:(d + 1) * P],
                                    ident32[:])
                t = singles.tile([P, NL], BF16, name=f"lmT{d}")
                nc.scalar.activation(t[:], ps_t[:], IDENT, scale=1.0 / SEG)
                if d == 0:
                    dump("lm_T0", t[:])
                lm_T.append(t)

            # K_lm_T [dout, 32] per dout tile (+ k bias); V_lm [32, 768] (+ v bias)
            klm_T = []
            for m in range(DT):
                ps_k = pss.tile([P, NL], F32, name=f"klm_ps{m}", tag="pss")
                for d in range(DT):
                    nc.tensor.matmul(ps_k[:], wsb["k", d][:, m * P:(m + 1) * P],
                                     lm_T[d][:], start=(d == 0), stop=(d == DT - 1))
                t = singles.tile([P, NL], BF16, name=f"klmT{m}")
                nc.scalar.activation(t[:], ps_k[:], IDENT, bias=kb_sb[:, m:m + 1])
                if m == 0:
                    dump("klm_T0", t[:])
                klm_T.append(t)
            ps_v1 = pss.tile([NL, 512], F32, name="ps_vlm1", tag="pss")
            ps_v2 = pss.tile([NL, 256], F32, name="ps_vlm2", tag="pss")
            for d in range(DT):
                nc.tensor.matmul(ps_v1[:], lm_T[d][:], wsb["v", d][:, 0:512],
                                 start=(d == 0), stop=(d == DT - 1))
                nc.tensor.matmul(ps_v2[:], lm_T[d][:], wsb["v", d][:, 512:D],
                                 start=(d == 0), stop=(d == DT - 1))
            vlm = singles.tile([NL, D], BF16, name="vlm")
            nc.vector.tensor_add(vlm[:, 0:512], ps_v1[:], vb_bc[0:NL, 0:512])
            nc.vector.tensor_add(vlm[:, 512:D], ps_v2[:], vb_bc[0:NL, 512:D])
            dump("vlm_d", vlm[:, :])

            # ---- main chunk pipeline (emission software-pipelined) -------
            def emit_proj(c):
                """QKV projections for chunk c; returns (qT, kT, vt)."""
                T0 = c * CS
                xt_c = []
                for d in range(DT):
                    t = sb.tile([P, CS], BF16, name=f"xt{c}_{d}", tag="xt",
                                bufs=14)
                    nc.sync.dma_start(t[:], xt_d[d * P:(d + 1) * P, T0:T0 + CS])
                    xt_c.append(t)
                qT, kT = [], []
                for m in range(DT):
                    ps_q = psa.tile([P, CS], F32, name=f"q_ps{c}_{m}", tag="psa")
                    for d in range(DT):
                        nc.tensor.matmul(ps_q[:], wsb["q", d][:, m * P:(m + 1) * P],
                                         xt_c[d][:], start=(d == 0),
                                         stop=(d == DT - 1))
                    t = sb.tile([P, CS], BF16, name=f"qT{c}_{m}", tag="qT", bufs=8)
                    nc.scalar.activation(t[:], ps_q[:], IDENT, scale=SCALE,
                                         bias=qb_sb[:, m:m + 1])
                    if c == 0 and m == 0:
                        dump("qT00", t[:])
                    qT.append(t)
                for m in range(DT):
                    ps_k = psa.tile([P, CS], F32, name=f"k_ps{c}_{m}", tag="psa")
                    for d in range(DT):
                        nc.tensor.matmul(ps_k[:], wsb["k", d][:, m * P:(m + 1) * P],
                                         xt_c[d][:], start=(d == 0),
                                         stop=(d == DT - 1))
                    t = sb.tile([P, CS], BF16, name=f"kT{c}_{m}", tag="kT", bufs=8)
                    nc.scalar.activation(t[:], ps_k[:], IDENT,
                                         bias=kb_sb[:, m:m + 1])
                    if c == 0 and m == 0:
                        dump("kT00", t[:])
                    kT.append(t)
                vt = []
                for tt in range(CS // P):
                    ps_1 = psa.tile([P, 512], F32, name=f"v1_ps{c}_{tt}", tag="psa")
                    ps_2 = psa.tile([P, 256], F32, name=f"v2_ps{c}_{tt}", tag="psa")
                    for d in range(DT):
                        nc.tensor.matmul(ps_1[:], xt_c[d][:, tt * P:(tt + 1) * P],
                                         wsb["v", d][:, 0:512], start=(d == 0),
                                         stop=(d == DT - 1))
                        nc.tensor.matmul(ps_2[:], xt_c[d][:, tt * P:(tt + 1) * P],
                                         wsb["v", d][:, 512:D], start=(d == 0),
                                         stop=(d == DT - 1))
                    t = sb.tile([P, D], BF16, name=f"v{c}_{tt}", tag="v", bufs=6)
                    nc.vector.tensor_add(t[:, 0:512], ps_1[:], vb_bc[:, 0:512])
                    nc.vector.tensor_add(t[:, 512:D], ps_2[:], vb_bc[:, 512:D])
                    if c == 0 and tt == 0:
                        dump("v00", t[:])
                    vt.append(t)
                return qT, kT, vt

            def emit_attn(c, qT, kT, vt):
                """Attention for chunk c; returns normalized oT tiles."""
                oT = []
                for p in range(DT):
                    ps_o = pso_pool.tile([P, CS], F32, name=f"o_ps{c}_{p}",
                                         tag="pso")
                    ps_z = psz_pool.tile([P, CS], F32, name=f"z_ps{c}_{p}",
                                         tag="psz")
                    for kt in range(5):
                        if kt == 0:
                            ksz = NL
                            k0 = klm_T[p][0:64, :]
                            k1 = klm_T[p][64:P, :]
                            v0 = vlm[0:NL, p * P: p * P + 64]
                            v1 = vlm[0:NL, p * P + 64: (p + 1) * P]
                        else:
                            j = kt - 1
                            ksz = P
                            k0 = kT[p][0:64, j * P:(j + 1) * P]
                            k1 = kT[p][64:P, j * P:(j + 1) * P]
                            v0 = vt[j][:, p * P: p * P + 64]
                            v1 = vt[j][:, p * P + 64: (p + 1) * P]
                        s0 = pss.tile([P, CS], F32, name=f"s0_{c}_{p}_{kt}",
                                      tag="pss")
                        s1 = pss.tile([P, CS], F32, name=f"s1_{c}_{p}_{kt}",
                                      tag="pss")
                        nc.tensor.matmul(s0[0:ksz, :], k0, qT[p][0:64, :],
                                         start=True, stop=True)
                        nc.tensor.matmul(s1[0:ksz, :], k1, qT[p][64:P, :],
                                         start=True, stop=True)
                        e0 = sb.tile([P, CS], BF16, name=f"e0_{c}_{p}_{kt}",
                                     tag="e", bufs=6)
                        e1 = sb.tile([P, CS], BF16, name=f"e1_{c}_{p}_{kt}",
                                     tag="e", bufs=6)
                        nc.scalar.activation(e0[0:ksz, :], s0[0:ksz, :], EXP)
                        nc.scalar.activation(e1[0:ksz, :], s1[0:ksz, :], EXP)
                        if c == 0 and p == 0 and kt == 0:
                            dump("e0_p0k0", e0[0:ksz, :])
                        if c == 0 and p == 0 and kt == 1:
                            dump("e0_p0k1", e0[0:ksz, :])
                            dump("e1_p0k1", e1[0:ksz, :])
                        nc.tensor.matmul(ps_o[0:64, :], v0[0:ksz, :], e0[0:ksz, :],
                                         start=(kt == 0), stop=(kt == 4),
                                         tile_position=(0, 0))
                        nc.tensor.matmul(ps_o[64:P, :], v1[0:ksz, :], e1[0:ksz, :],
                                         start=(kt == 0), stop=(kt == 4),
                                         tile_position=(0, 64))
                        nc.tensor.matmul(ps_z[0:1, :], ones[0:ksz, :], e0[0:ksz, :],
                                         start=(kt == 0), stop=(kt == 4),
                                         tile_position=(0, 0))
                        nc.tensor.matmul(ps_z[32:33, :], ones[0:ksz, :],
                                         e1[0:ksz, :], start=(kt == 0),
                                         stop=(kt == 4), tile_position=(0, 32))
                    if c == 0 and p == 0:
                        dump("z_p0", ps_z[:, :])
                    # evict unnormalized O immediately (frees psum for the PE);
                    # the normalize happens off the PE critical path.
                    oTu = sb.tile([P, CS], F32, name=f"oTu{c}_{p}", tag="oTu",
                                  bufs=4)
                    nc.scalar.copy(oTu[:], ps_o[:])
                    rz = sb.tile([33, CS], F32, name=f"rz{c}_{p}", tag="rz",
                                 bufs=3)
                    nc.vector.reciprocal(rz[0:1, :], ps_z[0:1, :])
                    nc.vector.reciprocal(rz[32:33, :], ps_z[32:33, :])
                    zdram = dram.tile([2, CS], F32, name=f"zd{c}_{p}", tag="zd",
                                      bufs=4)
                    nc.gpsimd.dma_start(zdram[0:1, :], rz[0:1, :])
                    nc.gpsimd.dma_start(zdram[1:2, :], rz[32:33, :])
                    rzb = sb.tile([P, CS], F32, name=f"rzb{c}_{p}", tag="rzb",
                                  bufs=3)
                    nc.gpsimd.dma_start(rzb[0:64, :],
                                         zdram[0:1, :].partition_broadcast(64))
                    nc.gpsimd.dma_start(rzb[64:P, :],
                                         zdram[1:2, :].partition_broadcast(64))
                    if c == 0 and p == 0:
                        dump("rzb_p0", rzb[:, :])
                    t = sb.tile([P, CS], BF16, name=f"oT{c}_{p}", tag="oT",
                                bufs=8)
                    nc.vector.tensor_mul(t[:], oTu[:], rzb[:])
                    if c == 0 and p == 0:
                        dump("oT00", t[:])
                    oT.append(t)
                return oT

            def emit_yproj(c, oT):
                T0 = c * CS
                for tt in range(CS // P):
                    ps_1 = psa.tile([P, 512], F32, name=f"y1_ps{c}_{tt}", tag="psa")
                    ps_2 = psa.tile([P, 256], F32, name=f"y2_ps{c}_{tt}", tag="psa")
                    for d in range(DT):
                        nc.tensor.matmul(ps_1[:], oT[d][:, tt * P:(tt + 1) * P],
                                         wsb["o", d][:, 0:512], start=(d == 0),
                                         stop=(d == DT - 1))
                        nc.tensor.matmul(ps_2[:], oT[d][:, tt * P:(tt + 1) * P],
                                         wsb["o", d][:, 512:D], start=(d == 0),
                                         stop=(d == DT - 1))
                    t = sb.tile([P, D], F32, name=f"y{c}_{tt}", tag="y", bufs=3)
                    nc.vector.tensor_add(t[:, 0:512], ps_1[:], ob_bc[:, 0:512])
                    nc.vector.tensor_add(t[:, 512:D], ps_2[:], ob_bc[:, 512:D])
                    nc.sync.dma_start(
                        y_d[T0 + tt * P: T0 + (tt + 1) * P, :], t[:])

            prev = None  # (c, oT) awaiting output projection
            for c in range(NCH):
                qT, kT, vt = emit_proj(c)
                if prev is not None:
                    emit_yproj(*prev)
                oT = emit_attn(c, qT, kT, vt)
                prev = (c, oT)
            emit_yproj(*prev)

    nc.finalize()
    return nc


def _get_nc(debug_dump=False):
    key = ("nc", debug_dump)
    if key not in _CACHE:
        _CACHE[key] = _build_nc(debug_dump)
    return _CACHE[key]


def _make_in_maps(x, q_w, q_b, k_w, k_b, v_w, v_b, o_w, o_b):
    x = np.asarray(x, np.float32)
    shared = {
        "wq": np.ascontiguousarray(np.asarray(q_w, np.float32).T.astype(NPBF)),
        "wk": np.ascontiguousarray(np.asarray(k_w, np.float32).T.astype(NPBF)),
        "wv": np.ascontiguousarray(np.asarray(v_w, np.float32).T.astype(NPBF)),
        "wo": np.ascontiguousarray(np.asarray(o_w, np.float32).T.astype(NPBF)),
        "qb": np.ascontiguousarray(
            (np.asarray(q_b, np.float32) * SCALE).reshape(DT, P).T),
        "kb": np.ascontiguousarray(np.asarray(k_b, np.float32).reshape(DT, P).T),
        "vb": np.asarray(v_b, np.float32).reshape(1, D).copy(),
        "ob": np.asarray(o_b, np.float32).reshape(1, D).copy(),
    }
    in_maps = []
    for c in range(N_CORES):
        b, h = divmod(c, 2)
        xs = x[b, h * TOK:(h + 1) * TOK, :]
        m = dict(shared)
        m["xt"] = np.ascontiguousarray(xs.T.astype(NPBF))
        m["xn"] = np.ascontiguousarray(xs.astype(NPBF))
        in_maps.append(m)
    return in_maps


def run(trace=False, trace_cores=None, debug_dump=False, **inputs):
    nc = _get_nc(debug_dump)
    in_maps = _make_in_maps(**inputs)
    res = bass_utils.run_bass_kernel_spmd(
        nc, in_maps, core_ids=list(range(N_CORES)), trace=trace,
        trace_cores=trace_cores)
    out = np.empty((B, S, D), np.float32)
    for c in range(N_CORES):
        b, h = divmod(c, 2)
        out[b, h * TOK:(h + 1) * TOK, :] = res.results[c]["y"]
    return out, res


def kernel(**inputs) -> np.ndarray:
    out, _ = run(trace=False, **inputs)
    return out


# revision 19
# speedup vs baseline: 1.1735x; 1.0366x over previous
"""Chunked local attention (landmark-augmented) for 8 Trainium2 NeuronCores.

Model (see reference): B=4, S=8192, D=768, H=12 heads of 64, chunk=512,
NL=32 landmark tokens = mean of 32 evenly spaced 256-token segments.
Every chunk attends over [32 landmarks ; its own 512 tokens].

Sharding: core c handles batch b=c//2, tokens [h*4096, (h+1)*4096), h=c%2.
Each core computes the 16 landmark partial means of its own half-sequence
and all-gathers the other 16 from its pair core ({2b, 2b+1}).

Layout strategy (matmuls in bf16, fp32 psum; weight loads hide via FWL):
  - host passes x shard both natural [4096,768] (landmark sums only) and
    transposed [768,4096] (everything else), plus W.T weights [din,dout],
    all pre-cast to bf16.
  - Q_T, K_T computed transposed [768, 512] per chunk; V natural [512, 768].
  - scores computed transposed [ctx, 512] so softmax-sum and attn@V both
    contract over ctx on the partition dim; Z row-sums via ones-matmuls
    packed into free column strips of the array.
  - attention out O_T [768, 512] feeds the output projection directly.
  - emission is software-pipelined: chunk c's output projection is emitted
    after chunk c+1's QKV so the in-order PE queue never stalls on the
    softmax-normalize DMA round trip.
"""

import sys

for _p in ("/opt/trn_rl_repo", "/root/.axon_site/_ro/trn_rl_repo"):
    if _p not in sys.path:
        sys.path.append(_p)

import numpy as np
import ml_dtypes

import concourse.bacc as bacc
import concourse.tile as tile
from concourse import mybir
from concourse import bass_utils

B, S, D = 4, 8192, 768
H, HD = 12, 64
CS, NL = 512, 32
N_CORES = 8
TOK = B * S // N_CORES      # 4096 tokens per core
NCH = TOK // CS             # 8 chunks per core
SEG = S // NL               # 256 tokens per landmark segment
NSEG_LOC = TOK // SEG       # 16 local segments per core
DT = D // 128               # 6 din/dout tiles
P = 128
SCALE = HD ** -0.5          # 0.125

F32 = mybir.dt.float32
BF16 = mybir.dt.bfloat16
NPBF = ml_dtypes.bfloat16
EXP = mybir.ActivationFunctionType.Exp
IDENT = mybir.ActivationFunctionType.Identity

_CACHE = {}


def _build_nc(debug_dump=False):
    nc = bacc.Bacc("TRN2", target_bir_lowering=False, debug=False,
                   num_devices=N_CORES)

    # ---- DRAM I/O --------------------------------------------------------
    xt_d = nc.dram_tensor("xt", [D, TOK], BF16, kind="ExternalInput")
    xn_d = nc.dram_tensor("xn", [TOK, D], BF16, kind="ExternalInput")
    w_d = {k: nc.dram_tensor(f"w{k}", [D, D], BF16, kind="ExternalInput")
           for k in "qkvo"}
    qb_d = nc.dram_tensor("qb", [P, DT], F32, kind="ExternalInput")
    kb_d = nc.dram_tensor("kb", [P, DT], F32, kind="ExternalInput")
    vb_d = nc.dram_tensor("vb", [1, D], F32, kind="ExternalInput")
    ob_d = nc.dram_tensor("ob", [1, D], F32, kind="ExternalInput")
    y_d = nc.dram_tensor("y", [TOK, D], F32, kind="ExternalOutput")

    dbg = {}

    with tile.TileContext(nc) as tc:
        with (
            tc.tile_pool(name="wpool", bufs=1) as wpool,
            tc.tile_pool(name="singles", bufs=1) as singles,
            tc.tile_pool(name="sb", bufs=2) as sb,
            tc.tile_pool(name="psa", bufs=2, space="PSUM") as psa,
            tc.tile_pool(name="pss", bufs=3, space="PSUM") as pss,
            tc.tile_pool(name="pso", bufs=2, space="PSUM") as pso_pool,
            tc.tile_pool(name="psz", bufs=1, space="PSUM") as psz_pool,
            tc.tile_pool(name="dram", bufs=1, space="DRAM") as dram,
        ):
            if debug_dump:
                for nm, shp in [("lm_loc", [NSEG_LOC, D]), ("lm_nat", [NL, D]),
                                ("lm_T0", [P, NL]), ("klm_T0", [P, NL]),
                                ("vlm_d", [NL, D]), ("qT00", [P, CS]),
                                ("kT00", [P, CS]), ("v00", [P, D]),
                                ("e0_p0k0", [NL, CS]), ("e0_p0k1", [P, CS]),
                                ("e1_p0k1", [P, CS]), ("z_p0", [P, CS]),
                                ("rzb_p0", [P, CS]), ("oT00", [P, CS])]:
                    dbg[nm] = nc.dram_tensor("dbg_" + nm, shp, F32,
                                             kind="ExternalOutput")

            def dump(nm, ap):
                if nm in dbg:
                    t = sb.tile(list(ap.shape), F32, name="dmp_" + nm,
                                tag="dmp", bufs=2)
                    nc.vector.tensor_copy(t[:], ap)
                    nc.sync.dma_start(
                        dbg[nm][tuple(slice(0, s) for s in ap.shape)], t[:])

            # ---- constants / weights ------------------------------------
            ones = singles.tile([P, 1], BF16, name="ones")
            nc.vector.memset(ones[:], 1.0)
            # indicator patterns for landmark sums: ind[:, 16*s + s] = 1
            ind = singles.tile([P, 16 * NSEG_LOC], BF16, name="ind")
            nc.vector.memset(ind[:], 0.0)
            for s in range(NSEG_LOC):
                nc.vector.memset(ind[:, 16 * s + s: 16 * s + s + 1], 1.0)
            ident32 = singles.tile([32, 32], F32, name="ident32")
            from concourse.masks import make_identity
            make_identity(nc, ident32[:])

            wsb = {}
            for k in "qkvo":
                for d in range(DT):
                    t = wpool.tile([P, D], BF16, name=f"w{k}{d}", tag=f"w{k}{d}")
                    nc.scalar.dma_start(t[:], w_d[k][d * P:(d + 1) * P, :])
                    wsb[k, d] = t
            qb_sb = singles.tile([P, DT], F32, name="qb_sb")
            kb_sb = singles.tile([P, DT], F32, name="kb_sb")
            nc.scalar.dma_start(qb_sb[:], qb_d[:, :])
            nc.scalar.dma_start(kb_sb[:], kb_d[:, :])
            vb_bc = singles.tile([P, D], F32, name="vb_bc")
            ob_bc = singles.tile([P, D], F32, name="ob_bc")
            nc.scalar.dma_start(vb_bc[:], vb_d[0:1, :].partition_broadcast(P))
            nc.scalar.dma_start(ob_bc[:], ob_d[0:1, :].partition_broadcast(P))

            # ---- landmark partial sums (natural layout, via indicators) --
            # landmark accumulators borrow the attention-output psum slots:
            # those are first needed by chunk-0 attention, which already
            # depends on the landmarks, so this adds no serialization.
            lm_ps = pso_pool.tile([NSEG_LOC, 512], F32, name="lm_ps", tag="pso")
            lm_ps2 = pso_pool.tile([NSEG_LOC, 256], F32, name="lm_ps2", tag="pso")
            for tt in range(TOK // P):  # 32 token tiles
                xn_t = sb.tile([P, D], BF16, name=f"xn{tt}", tag="v", bufs=6)
                nc.gpsimd.dma_start(xn_t[:], xn_d[tt * P:(tt + 1) * P, :])
                s = tt * P // SEG
                lhs = ind[:, 16 * s: 16 * (s + 1)]
                nc.tensor.matmul(lm_ps[:, :], lhs, xn_t[:, 0:512],
                                 start=(tt == 0), stop=(tt == TOK // P - 1))
                nc.tensor.matmul(lm_ps2[:, :], lhs, xn_t[:, 512:D],
                                 start=(tt == 0), stop=(tt == TOK // P - 1))

            lm_loc = sb.tile([NSEG_LOC, D], F32, name="lm_loc", tag="rz", bufs=3)
            nc.scalar.copy(lm_loc[:, 0:512], lm_ps[:, :])
            nc.scalar.copy(lm_loc[:, 512:D], lm_ps2[:, :])
            dump("lm_loc", lm_loc[:, :])
            lm_in_b = dram.tile([NSEG_LOC, D], F32, name="lm_in_b")
            lm_out_b = dram.tile([NL, D], F32, name="lm_out_b")
            nc.gpsimd.dma_start(lm_in_b[:, :], lm_loc[:, :])
            nc.gpsimd.collective_compute(
                "AllGather", mybir.AluOpType.bypass,
                replica_groups=[[0, 1], [2, 3], [4, 5], [6, 7]],
                ins=[lm_in_b.opt()], outs=[lm_out_b.opt()],
            )
            lm_nat = sb.tile([NL, D], F32, name="lm_nat", tag="rz", bufs=3)
            nc.gpsimd.dma_start(lm_nat[:], lm_out_b[:, :])
            dump("lm_nat", lm_nat[:, :])

            # transpose to lm_T [din, 32] per din tile, scaled by 1/SEG
            lm_T = []
            for d in range(DT):
                ps_t = pss.tile([P, NL], F32, name=f"lmT_ps{d}", tag="pss")
                nc.tensor.transpose(ps_t[:], lm_nat[:, d * P:(d + 1) * P],
                                    ident32[:])
                t = singles.tile([P, NL], BF16, name=f"lmT{d}")
                nc.scalar.activation(t[:], ps_t[:], IDENT, scale=1.0 / SEG)
                if d == 0:
                    dump("lm_T0", t[:])
                lm_T.append(t)

            # K_lm_T [dout, 32] per dout tile (+ k bias); V_lm [32, 768] (+ v bias)
            klm_T = []
            for m in range(DT):
                ps_k = pss.tile([P, NL], F32, name=f"klm_ps{m}", tag="pss")
                for d in range(DT):
                    nc.tensor.matmul(ps_k[:], wsb["k", d][:, m * P:(m + 1) * P],
                                     lm_T[d][:], start=(d == 0), stop=(d == DT - 1))
                t = singles.tile([P, NL], BF16, name=f"klmT{m}")
                nc.scalar.activation(t[:], ps_k[:], IDENT, bias=kb_sb[:, m:m + 1])
                if m == 0:
                    dump("klm_T0", t[:])
                klm_T.append(t)
            ps_v1 = pss.tile([NL, 512], F32, name="ps_vlm1", tag="pss")
            ps_v2 = pss.tile([NL, 256], F32, name="ps_vlm2", tag="pss")
            for d in range(DT):
                nc.tensor.matmul(ps_v1[:], lm_T[d][:], wsb["v", d][:, 0:512],
                                 start=(d == 0), stop=(d == DT - 1))
                nc.tensor.matmul(ps_v2[:], lm_T[d][:], wsb["v", d][:, 512:D],
                                 start=(d == 0), stop=(d == DT - 1))
            vlm = singles.tile([NL, D], BF16, name="vlm")
            nc.vector.tensor_add(vlm[:, 0:512], ps_v1[:], vb_bc[0:NL, 0:512])
            nc.vector.tensor_add(vlm[:, 512:D], ps_v2[:], vb_bc[0:NL, 512:D])
            dump("vlm_d", vlm[:, :])

            # ---- main chunk pipeline (emission software-pipelined) -------
            def emit_proj(c):
                """QKV projections for chunk c; returns (qT, kT, vt)."""
                T0 = c * CS
                xt_c = []
                for d in range(DT):
                    t = sb.tile([P, CS], BF16, name=f"xt{c}_{d}", tag="xt",
                                bufs=14)
                    nc.sync.dma_start(t[:], xt_d[d * P:(d + 1) * P, T0:T0 + CS])
                    xt_c.append(t)
                # V first: its psum slots are evicted by the (slower) DVE, and
                # doing it first lets those evictions overlap the Q/K matmuls,
                # so later psum reuse never waits on DVE.
                vt = []
                for tt in range(CS // P):
                    ps_1 = psa.tile([P, 512], F32, name=f"v1_ps{c}_{tt}", tag="psa")
                    ps_2 = psa.tile([P, 256], F32, name=f"v2_ps{c}_{tt}", tag="psa")
                    for d in range(DT):
                        nc.tensor.matmul(ps_1[:], xt_c[d][:, tt * P:(tt + 1) * P],
                                         wsb["v", d][:, 0:512], start=(d == 0),
                                         stop=(d == DT - 1))
                        nc.tensor.matmul(ps_2[:], xt_c[d][:, tt * P:(tt + 1) * P],
                                         wsb["v", d][:, 512:D], start=(d == 0),
                                         stop=(d == DT - 1))
                    t = sb.tile([P, D], BF16, name=f"v{c}_{tt}", tag="v", bufs=6)
                    nc.vector.tensor_add(t[:, 0:512], ps_1[:], vb_bc[:, 0:512])
                    nc.vector.tensor_add(t[:, 512:D], ps_2[:], vb_bc[:, 512:D])
                    if c == 0 and tt == 0:
                        dump("v00", t[:])
                    vt.append(t)
                qT, kT = [], []
                for m in range(DT):
                    ps_q = psa.tile([P, CS], F32, name=f"q_ps{c}_{m}", tag="psa")
                    for d in range(DT):
                        nc.tensor.matmul(ps_q[:], wsb["q", d][:, m * P:(m + 1) * P],
                                         xt_c[d][:], start=(d == 0),
                                         stop=(d == DT - 1))
                    t = sb.tile([P, CS], BF16, name=f"qT{c}_{m}", tag="qT", bufs=8)
                    nc.scalar.activation(t[:], ps_q[:], IDENT, scale=SCALE,
                                         bias=qb_sb[:, m:m + 1])
                    if c == 0 and m == 0:
                        dump("qT00", t[:])
                    qT.append(t)
                for m in range(DT):
                    ps_k = psa.tile([P, CS], F32, name=f"k_ps{c}_{m}", tag="psa")
                    for d in range(DT):
                        nc.tensor.matmul(ps_k[:], wsb["k", d][:, m * P:(m + 1) * P],
                                         xt_c[d][:], start=(d == 0),
                                         stop=(d == DT - 1))
                    t = sb.tile([P, CS], BF16, name=f"kT{c}_{m}", tag="kT", bufs=8)
                    nc.scalar.activation(t[:], ps_k[:], IDENT,
                                         bias=kb_sb[:, m:m + 1])
                    if c == 0 and m == 0:
                        dump("kT00", t[:])
                    kT.append(t)
                return qT, kT, vt

            def emit_scores(c, p, qT, kT):
                """Scores + exp for head pair p; returns the 10 e tiles."""
                es = []
                for kt in range(5):
                    if kt == 0:
                        ksz = NL
                        k0 = klm_T[p][0:64, :]
                        k1 = klm_T[p][64:P, :]
                    else:
                        j = kt - 1
                        ksz = P
                        k0 = kT[p][0:64, j * P:(j + 1) * P]
                        k1 = kT[p][64:P, j * P:(j + 1) * P]
                    s0 = pss.tile([P, CS], F32, name=f"s0_{c}_{p}_{kt}",
                                  tag="pss")
                    s1 = pss.tile([P, CS], F32, name=f"s1_{c}_{p}_{kt}",
                                  tag="pss")
                    nc.tensor.matmul(s0[0:ksz, :], k0, qT[p][0:64, :],
                                     start=True, stop=True)
                    nc.tensor.matmul(s1[0:ksz, :], k1, qT[p][64:P, :],
                                     start=True, stop=True)
                    e0 = sb.tile([P, CS], BF16, name=f"e0_{c}_{p}_{kt}",
                                 tag="e", bufs=22)
                    e1 = sb.tile([P, CS], BF16, name=f"e1_{c}_{p}_{kt}",
                                 tag="e", bufs=22)
                    nc.scalar.activation(e0[0:ksz, :], s0[0:ksz, :], EXP)
                    nc.scalar.activation(e1[0:ksz, :], s1[0:ksz, :], EXP)
                    if c == 0 and p == 0 and kt == 0:
                        dump("e0_p0k0", e0[0:ksz, :])
                    if c == 0 and p == 0 and kt == 1:
                        dump("e0_p0k1", e0[0:ksz, :])
                        dump("e1_p0k1", e1[0:ksz, :])
                    es.append((e0, e1, ksz))
                return es

            def emit_attnv(c, p, es, vt, oT):
                """attn@V + softmax denominator + normalize for pair p."""
                ps_o = pso_pool.tile([P, CS], F32, name=f"o_ps{c}_{p}",
                                     tag="pso")
                ps_z = psz_pool.tile([P, CS], F32, name=f"z_ps{c}_{p}",
                                     tag="psz")
                for kt, (e0, e1, ksz) in enumerate(es):
                    if kt == 0:
                        v0 = vlm[0:NL, p * P: p * P + 64]
                        v1 = vlm[0:NL, p * P + 64: (p + 1) * P]
                    else:
                        j = kt - 1
                        v0 = vt[j][:, p * P: p * P + 64]
                        v1 = vt[j][:, p * P + 64: (p + 1) * P]
                    nc.tensor.matmul(ps_o[0:64, :], v0[0:ksz, :], e0[0:ksz, :],
                                     start=(kt == 0), stop=(kt == 4),
                                     tile_position=(0, 0))
                    nc.tensor.matmul(ps_o[64:P, :], v1[0:ksz, :], e1[0:ksz, :],
                                     start=(kt == 0), stop=(kt == 4),
                                     tile_position=(0, 64))
                    nc.tensor.matmul(ps_z[0:1, :], ones[0:ksz, :], e0[0:ksz, :],
                                     start=(kt == 0), stop=(kt == 4),
                                     tile_position=(0, 0))
                    nc.tensor.matmul(ps_z[32:33, :], ones[0:ksz, :],
                                     e1[0:ksz, :], start=(kt == 0),
                                     stop=(kt == 4), tile_position=(0, 32))
                if c == 0 and p == 0:
                    dump("z_p0", ps_z[:, :])
                # evict unnormalized O immediately (frees psum for the PE);
                # the normalize happens off the PE critical path.
                oTu = sb.tile([P, CS], F32, name=f"oTu{c}_{p}", tag="oTu",
                              bufs=4)
                nc.scalar.copy(oTu[:], ps_o[:])
                rz = sb.tile([33, CS], F32, name=f"rz{c}_{p}", tag="rz",
                             bufs=3)
                nc.vector.reciprocal(rz[0:1, :], ps_z[0:1, :])
                nc.vector.reciprocal(rz[32:33, :], ps_z[32:33, :])
                zdram = dram.tile([2, CS], F32, name=f"zd{c}_{p}", tag="zd",
                                  bufs=4)
                nc.gpsimd.dma_start(zdram[0:1, :], rz[0:1, :])
                nc.gpsimd.dma_start(zdram[1:2, :], rz[32:33, :])
                rzb = sb.tile([P, CS], F32, name=f"rzb{c}_{p}", tag="rzb",
                              bufs=3)
                nc.gpsimd.dma_start(rzb[0:64, :],
                                    zdram[0:1, :].partition_broadcast(64))
                nc.gpsimd.dma_start(rzb[64:P, :],
                                    zdram[1:2, :].partition_broadcast(64))
                if c == 0 and p == 0:
                    dump("rzb_p0", rzb[:, :])
                t = sb.tile([P, CS], BF16, name=f"oT{c}_{p}", tag="oT",
                            bufs=8)
                nc.vector.tensor_mul(t[:], oTu[:], rzb[:])
                if c == 0 and p == 0:
                    dump("oT00", t[:])
                oT.append(t)

            def emit_yproj(c, oT):
                T0 = c * CS
                for tt in range(CS // P):
                    ps_1 = psa.tile([P, 512], F32, name=f"y1_ps{c}_{tt}", tag="psa")
                    ps_2 = psa.tile([P, 256], F32, name=f"y2_ps{c}_{tt}", tag="psa")
                    for d in range(DT):
                        nc.tensor.matmul(ps_1[:], oT[d][:, tt * P:(tt + 1) * P],
                                         wsb["o", d][:, 0:512], start=(d == 0),
                                         stop=(d == DT - 1))
                        nc.tensor.matmul(ps_2[:], oT[d][:, tt * P:(tt + 1) * P],
                                         wsb["o", d][:, 512:D], start=(d == 0),
                                         stop=(d == DT - 1))
                    t = sb.tile([P, D], F32, name=f"y{c}_{tt}", tag="y", bufs=3)
                    nc.vector.tensor_add(t[:, 0:512], ps_1[:], ob_bc[:, 0:512])
                    nc.vector.tensor_add(t[:, 512:D], ps_2[:], ob_bc[:, 512:D])
                    nc.sync.dma_start(
                        y_d[T0 + tt * P: T0 + (tt + 1) * P, :], t[:])

            # software-pipelined emission: within a chunk, the scores of pair
            # p are emitted before attn@V of pair p-1, so the PE always has a
            # dense stream of ready work while the exps for the younger pair
            # run on the scalar engine. Output projection lags one chunk.
            prev = None  # (c, oT) awaiting output projection
            for c in range(NCH):
                qT, kT, vt = emit_proj(c)
                if prev is not None:
                    emit_yproj(*prev)
                oT = []
                pend = None  # pair whose scores are done, attnV pending
                for p in range(DT):
                    es = emit_scores(c, p, qT, kT)
                    if pend is not None:
                        emit_attnv(c, pend[0], pend[1], vt, oT)
                    pend = (p, es)
                emit_attnv(c, pend[0], pend[1], vt, oT)
                prev = (c, oT)
            emit_yproj(*prev)

    nc.finalize()
    return nc


def _get_nc(debug_dump=False):
    key = ("nc", debug_dump)
    if key not in _CACHE:
        _CACHE[key] = _build_nc(debug_dump)
    return _CACHE[key]


def _make_in_maps(x, q_w, q_b, k_w, k_b, v_w, v_b, o_w, o_b):
    x = np.asarray(x, np.float32)
    shared = {
        "wq": np.ascontiguousarray(np.asarray(q_w, np.float32).T.astype(NPBF)),
        "wk": np.ascontiguousarray(np.asarray(k_w, np.float32).T.astype(NPBF)),
        "wv": np.ascontiguousarray(np.asarray(v_w, np.float32).T.astype(NPBF)),
        "wo": np.ascontiguousarray(np.asarray(o_w, np.float32).T.astype(NPBF)),
        "qb": np.ascontiguousarray(
            (np.asarray(q_b, np.float32) * SCALE).reshape(DT, P).T),
        "kb": np.ascontiguousarray(np.asarray(k_b, np.float32).reshape(DT, P).T),
        "vb": np.asarray(v_b, np.float32).reshape(1, D).copy(),
        "ob": np.asarray(o_b, np.float32).reshape(1, D).copy(),
    }
    in_maps = []
    for c in range(N_CORES):
        b, h = divmod(c, 2)
        xs = x[b, h * TOK:(h + 1) * TOK, :]
        m = dict(shared)
        m["xt"] = np.ascontiguousarray(xs.T.astype(NPBF))
        m["xn"] = np.ascontiguousarray(xs.astype(NPBF))
        in_maps.append(m)
    return in_maps


def run(trace=False, trace_cores=None, debug_dump=False, **inputs):
    nc = _get_nc(debug_dump)
    in_maps = _make_in_maps(**inputs)
    res = bass_utils.run_bass_kernel_spmd(
        nc, in_maps, core_ids=list(range(N_CORES)), trace=trace,
        trace_cores=trace_cores)
    out = np.empty((B, S, D), np.float32)
    for c in range(N_CORES):
        b, h = divmod(c, 2)
        out[b, h * TOK:(h + 1) * TOK, :] = res.results[c]["y"]
    return out, res


def kernel(**inputs) -> np.ndarray:
    out, _ = run(trace=False, **inputs)
    return out


# revision 21
# speedup vs baseline: 1.1857x; 1.0103x over previous
"""Chunked local attention (landmark-augmented) for 8 Trainium2 NeuronCores.

Model (see reference): B=4, S=8192, D=768, H=12 heads of 64, chunk=512,
NL=32 landmark tokens = mean of 32 evenly spaced 256-token segments.
Every chunk attends over [32 landmarks ; its own 512 tokens].

Sharding: core c handles batch b=c//2, tokens [h*4096, (h+1)*4096), h=c%2.
Each core computes the 16 landmark partial means of its own half-sequence
and all-gathers the other 16 from its pair core ({2b, 2b+1}).

Layout strategy (matmuls in bf16, fp32 psum; weight loads hide via FWL):
  - host passes x shard both natural [4096,768] (landmark sums only) and
    transposed [768,4096] (everything else), plus W.T weights [din,dout],
    all pre-cast to bf16.
  - Q_T, K_T computed transposed [768, 512] per chunk; V natural [512, 768].
  - scores computed transposed [ctx, 512] so softmax-sum and attn@V both
    contract over ctx on the partition dim; Z row-sums via ones-matmuls
    packed into free column strips of the array.
  - attention out O_T [768, 512] feeds the output projection directly.
  - emission is software-pipelined: chunk c's output projection is emitted
    after chunk c+1's QKV so the in-order PE queue never stalls on the
    softmax-normalize DMA round trip.
"""

import sys

for _p in ("/opt/trn_rl_repo", "/root/.axon_site/_ro/trn_rl_repo"):
    if _p not in sys.path:
        sys.path.append(_p)

import numpy as np
import ml_dtypes

import concourse.bacc as bacc
import concourse.tile as tile
from concourse import mybir
from concourse import bass_utils

B, S, D = 4, 8192, 768
H, HD = 12, 64
CS, NL = 512, 32
N_CORES = 8
TOK = B * S // N_CORES      # 4096 tokens per core
NCH = TOK // CS             # 8 chunks per core
SEG = S // NL               # 256 tokens per landmark segment
NSEG_LOC = TOK // SEG       # 16 local segments per core
DT = D // 128               # 6 din/dout tiles
P = 128
SCALE = HD ** -0.5          # 0.125

F32 = mybir.dt.float32
BF16 = mybir.dt.bfloat16
NPBF = ml_dtypes.bfloat16
EXP = mybir.ActivationFunctionType.Exp
IDENT = mybir.ActivationFunctionType.Identity

_CACHE = {}


def _build_nc(debug_dump=False):
    nc = bacc.Bacc("TRN2", target_bir_lowering=False, debug=False,
                   num_devices=N_CORES)

    # ---- DRAM I/O --------------------------------------------------------
    xt_d = nc.dram_tensor("xt", [D, TOK], BF16, kind="ExternalInput")
    xn_d = nc.dram_tensor("xn", [TOK, D], BF16, kind="ExternalInput")
    w_d = {k: nc.dram_tensor(f"w{k}", [D, D], BF16, kind="ExternalInput")
           for k in "qkvo"}
    qb_d = nc.dram_tensor("qb", [P, DT], F32, kind="ExternalInput")
    kb_d = nc.dram_tensor("kb", [P, DT], F32, kind="ExternalInput")
    vb_d = nc.dram_tensor("vb", [1, D], F32, kind="ExternalInput")
    ob_d = nc.dram_tensor("ob", [1, D], F32, kind="ExternalInput")
    y_d = nc.dram_tensor("y", [TOK, D], F32, kind="ExternalOutput")

    dbg = {}

    with tile.TileContext(nc) as tc:
        with (
            tc.tile_pool(name="wpool", bufs=1) as wpool,
            tc.tile_pool(name="singles", bufs=1) as singles,
            tc.tile_pool(name="sb", bufs=2) as sb,
            tc.tile_pool(name="psa", bufs=3, space="PSUM") as psa,
            tc.tile_pool(name="pss", bufs=3, space="PSUM") as pss,
            tc.tile_pool(name="pso", bufs=1, space="PSUM") as pso_pool,
            tc.tile_pool(name="psz", bufs=1, space="PSUM") as psz_pool,
            tc.tile_pool(name="dram", bufs=1, space="DRAM") as dram,
        ):
            if debug_dump:
                for nm, shp in [("lm_loc", [NSEG_LOC, D]), ("lm_nat", [NL, D]),
                                ("lm_T0", [P, NL]), ("klm_T0", [P, NL]),
                                ("vlm_d", [NL, D]), ("qT00", [P, CS]),
                                ("kT00", [P, CS]), ("v00", [P, D]),
                                ("e0_p0k0", [NL, CS]), ("e0_p0k1", [P, CS]),
                                ("e1_p0k1", [P, CS]), ("z_p0", [P, CS]),
                                ("rzb_p0", [P, CS]), ("oT00", [P, CS])]:
                    dbg[nm] = nc.dram_tensor("dbg_" + nm, shp, F32,
                                             kind="ExternalOutput")

            def dump(nm, ap):
                if nm in dbg:
                    t = sb.tile(list(ap.shape), F32, name="dmp_" + nm,
                                tag="dmp", bufs=2)
                    nc.vector.tensor_copy(t[:], ap)
                    nc.sync.dma_start(
                        dbg[nm][tuple(slice(0, s) for s in ap.shape)], t[:])

            # ---- constants / weights ------------------------------------
            ones = singles.tile([P, 1], BF16, name="ones")
            nc.vector.memset(ones[:], 1.0)
            # indicator patterns for landmark sums: ind[:, 16*s + s] = 1
            ind = singles.tile([P, 16 * NSEG_LOC], BF16, name="ind")
            nc.vector.memset(ind[:], 0.0)
            for s in range(NSEG_LOC):
                nc.vector.memset(ind[:, 16 * s + s: 16 * s + s + 1], 1.0)
            ident32 = singles.tile([32, 32], F32, name="ident32")
            from concourse.masks import make_identity
            make_identity(nc, ident32[:])

            wsb = {}
            for k in "qkvo":
                for d in range(DT):
                    t = wpool.tile([P, D], BF16, name=f"w{k}{d}", tag=f"w{k}{d}")
                    nc.scalar.dma_start(t[:], w_d[k][d * P:(d + 1) * P, :])
                    wsb[k, d] = t
            qb_sb = singles.tile([P, DT], F32, name="qb_sb")
            kb_sb = singles.tile([P, DT], F32, name="kb_sb")
            nc.scalar.dma_start(qb_sb[:], qb_d[:, :])
            nc.scalar.dma_start(kb_sb[:], kb_d[:, :])
            vb_bc = singles.tile([P, D], F32, name="vb_bc")
            ob_bc = singles.tile([P, D], F32, name="ob_bc")
            nc.scalar.dma_start(vb_bc[:], vb_d[0:1, :].partition_broadcast(P))
            nc.scalar.dma_start(ob_bc[:], ob_d[0:1, :].partition_broadcast(P))

            # ---- landmark partial sums (natural layout, via indicators) --
            # landmark accumulators borrow the attention-output psum slots:
            # those are first needed by chunk-0 attention, which already
            # depends on the landmarks, so this adds no serialization.
            lm_ps = pso_pool.tile([NSEG_LOC, 512], F32, name="lm_ps", tag="pso")
            lm_ps2 = psz_pool.tile([NSEG_LOC, 256], F32, name="lm_ps2", tag="psz")
            for tt in range(TOK // P):  # 32 token tiles
                xn_t = sb.tile([P, D], BF16, name=f"xn{tt}", tag="v", bufs=6)
                nc.gpsimd.dma_start(xn_t[:], xn_d[tt * P:(tt + 1) * P, :])
                s = tt * P // SEG
                lhs = ind[:, 16 * s: 16 * (s + 1)]
                nc.tensor.matmul(lm_ps[:, :], lhs, xn_t[:, 0:512],
                                 start=(tt == 0), stop=(tt == TOK // P - 1))
                nc.tensor.matmul(lm_ps2[:, :], lhs, xn_t[:, 512:D],
                                 start=(tt == 0), stop=(tt == TOK // P - 1))

            lm_loc = sb.tile([NSEG_LOC, D], F32, name="lm_loc", tag="rz", bufs=3)
            nc.scalar.copy(lm_loc[:, 0:512], lm_ps[:, :])
            nc.scalar.copy(lm_loc[:, 512:D], lm_ps2[:, :])
            dump("lm_loc", lm_loc[:, :])
            lm_in_b = dram.tile([NSEG_LOC, D], F32, name="lm_in_b")
            lm_out_b = dram.tile([NL, D], F32, name="lm_out_b")
            nc.gpsimd.dma_start(lm_in_b[:, :], lm_loc[:, :])
            nc.gpsimd.collective_compute(
                "AllGather", mybir.AluOpType.bypass,
                replica_groups=[[0, 1], [2, 3], [4, 5], [6, 7]],
                ins=[lm_in_b.opt()], outs=[lm_out_b.opt()],
            )
            lm_nat = sb.tile([NL, D], F32, name="lm_nat", tag="rz", bufs=3)
            nc.gpsimd.dma_start(lm_nat[:], lm_out_b[:, :])
            dump("lm_nat", lm_nat[:, :])

            # transpose to lm_T [din, 32] per din tile, scaled by 1/SEG
            lm_T = []
            for d in range(DT):
                ps_t = pss.tile([P, NL], F32, name=f"lmT_ps{d}", tag="pss")
                nc.tensor.transpose(ps_t[:], lm_nat[:, d * P:(d + 1) * P],
                                    ident32[:])
                t = singles.tile([P, NL], BF16, name=f"lmT{d}")
                nc.scalar.activation(t[:], ps_t[:], IDENT, scale=1.0 / SEG)
                if d == 0:
                    dump("lm_T0", t[:])
                lm_T.append(t)

            # K_lm_T [dout, 32] per dout tile (+ k bias); V_lm [32, 768] (+ v bias)
            klm_T = []
            for m in range(DT):
                ps_k = pss.tile([P, NL], F32, name=f"klm_ps{m}", tag="pss")
                for d in range(DT):
                    nc.tensor.matmul(ps_k[:], wsb["k", d][:, m * P:(m + 1) * P],
                                     lm_T[d][:], start=(d == 0), stop=(d == DT - 1))
                t = singles.tile([P, NL], BF16, name=f"klmT{m}")
                nc.scalar.activation(t[:], ps_k[:], IDENT, bias=kb_sb[:, m:m + 1])
                if m == 0:
                    dump("klm_T0", t[:])
                klm_T.append(t)
            ps_v1 = pss.tile([NL, 512], F32, name="ps_vlm1", tag="pss")
            ps_v2 = pss.tile([NL, 256], F32, name="ps_vlm2", tag="pss")
            for d in range(DT):
                nc.tensor.matmul(ps_v1[:], lm_T[d][:], wsb["v", d][:, 0:512],
                                 start=(d == 0), stop=(d == DT - 1))
                nc.tensor.matmul(ps_v2[:], lm_T[d][:], wsb["v", d][:, 512:D],
                                 start=(d == 0), stop=(d == DT - 1))
            vlm = singles.tile([NL, D], BF16, name="vlm")
            nc.vector.tensor_add(vlm[:, 0:512], ps_v1[:], vb_bc[0:NL, 0:512])
            nc.vector.tensor_add(vlm[:, 512:D], ps_v2[:], vb_bc[0:NL, 512:D])
            dump("vlm_d", vlm[:, :])

            # ---- main chunk pipeline (emission software-pipelined) -------
            def emit_proj(c):
                """QKV projections for chunk c; returns (qT, kT, vt)."""
                T0 = c * CS
                xt_c = []
                for d in range(DT):
                    t = sb.tile([P, CS], BF16, name=f"xt{c}_{d}", tag="xt",
                                bufs=14)
                    nc.sync.dma_start(t[:], xt_d[d * P:(d + 1) * P, T0:T0 + CS])
                    xt_c.append(t)
                # V first: its psum slots are evicted by the (slower) DVE, and
                # doing it first lets those evictions overlap the Q/K matmuls,
                # so later psum reuse never waits on DVE.
                vt = []
                for tt in range(CS // P):
                    ps_1 = psa.tile([P, 512], F32, name=f"v1_ps{c}_{tt}", tag="psa")
                    ps_2 = psa.tile([P, 256], F32, name=f"v2_ps{c}_{tt}", tag="psa")
                    for d in range(DT):
                        nc.tensor.matmul(ps_1[:], xt_c[d][:, tt * P:(tt + 1) * P],
                                         wsb["v", d][:, 0:512], start=(d == 0),
                                         stop=(d == DT - 1))
                        nc.tensor.matmul(ps_2[:], xt_c[d][:, tt * P:(tt + 1) * P],
                                         wsb["v", d][:, 512:D], start=(d == 0),
                                         stop=(d == DT - 1))
                    t = sb.tile([P, D], BF16, name=f"v{c}_{tt}", tag="v", bufs=6)
                    nc.vector.tensor_add(t[:, 0:512], ps_1[:], vb_bc[:, 0:512])
                    nc.vector.tensor_add(t[:, 512:D], ps_2[:], vb_bc[:, 512:D])
                    if c == 0 and tt == 0:
                        dump("v00", t[:])
                    vt.append(t)
                qT, kT = [], []
                for m in range(DT):
                    ps_q = psa.tile([P, CS], F32, name=f"q_ps{c}_{m}", tag="psa")
                    for d in range(DT):
                        nc.tensor.matmul(ps_q[:], wsb["q", d][:, m * P:(m + 1) * P],
                                         xt_c[d][:], start=(d == 0),
                                         stop=(d == DT - 1))
                    t = sb.tile([P, CS], BF16, name=f"qT{c}_{m}", tag="qT", bufs=8)
                    nc.vector.tensor_scalar_add(t[:], ps_q[:], qb_sb[:, m:m + 1])
                    if c == 0 and m == 0:
                        dump("qT00", t[:])
                    qT.append(t)
                for m in range(DT):
                    ps_k = psa.tile([P, CS], F32, name=f"k_ps{c}_{m}", tag="psa")
                    for d in range(DT):
                        nc.tensor.matmul(ps_k[:], wsb["k", d][:, m * P:(m + 1) * P],
                                         xt_c[d][:], start=(d == 0),
                                         stop=(d == DT - 1))
                    t = sb.tile([P, CS], BF16, name=f"kT{c}_{m}", tag="kT", bufs=8)
                    nc.vector.tensor_scalar_add(t[:], ps_k[:], kb_sb[:, m:m + 1])
                    if c == 0 and m == 0:
                        dump("kT00", t[:])
                    kT.append(t)
                return qT, kT, vt

            def emit_scores(c, p, qT, kT):
                """Scores + exp for head pair p; returns the 10 e tiles."""
                es = []
                for kt in range(5):
                    if kt == 0:
                        ksz = NL
                        k0 = klm_T[p][0:64, :]
                        k1 = klm_T[p][64:P, :]
                    else:
                        j = kt - 1
                        ksz = P
                        k0 = kT[p][0:64, j * P:(j + 1) * P]
                        k1 = kT[p][64:P, j * P:(j + 1) * P]
                    s0 = pss.tile([P, CS], F32, name=f"s0_{c}_{p}_{kt}",
                                  tag="pss")
                    s1 = pss.tile([P, CS], F32, name=f"s1_{c}_{p}_{kt}",
                                  tag="pss")
                    nc.tensor.matmul(s0[0:ksz, :], k0, qT[p][0:64, :],
                                     start=True, stop=True)
                    nc.tensor.matmul(s1[0:ksz, :], k1, qT[p][64:P, :],
                                     start=True, stop=True)
                    e0 = sb.tile([P, CS], BF16, name=f"e0_{c}_{p}_{kt}",
                                 tag="e", bufs=22)
                    e1 = sb.tile([P, CS], BF16, name=f"e1_{c}_{p}_{kt}",
                                 tag="e", bufs=22)
                    nc.scalar.activation(e0[0:ksz, :], s0[0:ksz, :], EXP)
                    nc.scalar.activation(e1[0:ksz, :], s1[0:ksz, :], EXP)
                    if c == 0 and p == 0 and kt == 0:
                        dump("e0_p0k0", e0[0:ksz, :])
                    if c == 0 and p == 0 and kt == 1:
                        dump("e0_p0k1", e0[0:ksz, :])
                        dump("e1_p0k1", e1[0:ksz, :])
                    es.append((e0, e1, ksz))
                return es

            def emit_attnv(c, p, es, vt, oT):
                """attn@V + softmax denominator + normalize for pair p."""
                ps_o = pso_pool.tile([P, CS], F32, name=f"o_ps{c}_{p}",
                                     tag="pso")
                ps_z = psz_pool.tile([P, CS], F32, name=f"z_ps{c}_{p}",
                                     tag="psz")
                for kt, (e0, e1, ksz) in enumerate(es):
                    if kt == 0:
                        v0 = vlm[0:NL, p * P: p * P + 64]
                        v1 = vlm[0:NL, p * P + 64: (p + 1) * P]
                    else:
                        j = kt - 1
                        v0 = vt[j][:, p * P: p * P + 64]
                        v1 = vt[j][:, p * P + 64: (p + 1) * P]
                    nc.tensor.matmul(ps_o[0:64, :], v0[0:ksz, :], e0[0:ksz, :],
                                     start=(kt == 0), stop=(kt == 4),
                                     tile_position=(0, 0))
                    nc.tensor.matmul(ps_o[64:P, :], v1[0:ksz, :], e1[0:ksz, :],
                                     start=(kt == 0), stop=(kt == 4),
                                     tile_position=(0, 64))
                    nc.tensor.matmul(ps_z[0:1, :], ones[0:ksz, :], e0[0:ksz, :],
                                     start=(kt == 0), stop=(kt == 4),
                                     tile_position=(0, 0))
                    nc.tensor.matmul(ps_z[32:33, :], ones[0:ksz, :],
                                     e1[0:ksz, :], start=(kt == 0),
                                     stop=(kt == 4), tile_position=(0, 32))
                if c == 0 and p == 0:
                    dump("z_p0", ps_z[:, :])
                # free the Z bank first (recips), then the O bank (copy); the
                # normalize itself is deferred so the denominator's DMA
                # broadcast round-trip never blocks the DVE queue.
                rz = sb.tile([33, CS], F32, name=f"rz{c}_{p}", tag="rz",
                             bufs=3)
                nc.vector.reciprocal(rz[0:1, :], ps_z[0:1, :])
                nc.vector.reciprocal(rz[32:33, :], ps_z[32:33, :])
                oTu = sb.tile([P, CS], F32, name=f"oTu{c}_{p}", tag="oTu",
                              bufs=4)
                nc.vector.tensor_copy(oTu[:], ps_o[:])
                zdram = dram.tile([2, CS], F32, name=f"zd{c}_{p}", tag="zd",
                                  bufs=4)
                nc.gpsimd.dma_start(zdram[0:1, :], rz[0:1, :])
                nc.gpsimd.dma_start(zdram[1:2, :], rz[32:33, :])
                rzb = sb.tile([P, CS], F32, name=f"rzb{c}_{p}", tag="rzb",
                              bufs=3)
                nc.gpsimd.dma_start(rzb[0:64, :],
                                    zdram[0:1, :].partition_broadcast(64))
                nc.gpsimd.dma_start(rzb[64:P, :],
                                    zdram[1:2, :].partition_broadcast(64))
                if c == 0 and p == 0:
                    dump("rzb_p0", rzb[:, :])
                t = sb.tile([P, CS], BF16, name=f"oT{c}_{p}", tag="oT",
                            bufs=8)
                oT.append(t)

                def mul(t=t, oTu=oTu, rzb=rzb, first=(c == 0 and p == 0)):
                    nc.vector.tensor_mul(t[:], oTu[:], rzb[:])
                    if first:
                        dump("oT00", t[:])
                return mul

            def emit_yproj(c, oT):
                T0 = c * CS
                for tt in range(CS // P):
                    ps_1 = psa.tile([P, 512], F32, name=f"y1_ps{c}_{tt}", tag="psa")
                    ps_2 = psa.tile([P, 256], F32, name=f"y2_ps{c}_{tt}", tag="psa")
                    for d in range(DT):
                        nc.tensor.matmul(ps_1[:], oT[d][:, tt * P:(tt + 1) * P],
                                         wsb["o", d][:, 0:512], start=(d == 0),
                                         stop=(d == DT - 1))
                        nc.tensor.matmul(ps_2[:], oT[d][:, tt * P:(tt + 1) * P],
                                         wsb["o", d][:, 512:D], start=(d == 0),
                                         stop=(d == DT - 1))
                    t = sb.tile([P, D], F32, name=f"y{c}_{tt}", tag="y", bufs=3)
                    nc.vector.tensor_add(t[:, 0:512], ps_1[:], ob_bc[:, 0:512])
                    nc.vector.tensor_add(t[:, 512:D], ps_2[:], ob_bc[:, 512:D])
                    nc.sync.dma_start(
                        y_d[T0 + tt * P: T0 + (tt + 1) * P, :], t[:])

            # software-pipelined emission: within a chunk, the scores of pair
            # p are emitted before attn@V of pair p-1, so the PE always has a
            # dense stream of ready work while the exps for the younger pair
            # run on the scalar engine. Output projection lags one chunk.
            prev = None  # (c, oT) awaiting output projection
            tail_muls = []
            for c in range(NCH):
                qT, kT, vt = emit_proj(c)
                for m in tail_muls:
                    m()
                tail_muls = []
                if prev is not None:
                    emit_yproj(*prev)
                oT = []
                pend = None   # pair whose scores are done, attnV pending
                muls = []     # deferred normalizes, flushed one pair late
                for p in range(DT):
                    es = emit_scores(c, p, qT, kT)
                    if pend is not None:
                        muls.append(emit_attnv(c, pend[0], pend[1], vt, oT))
                        if len(muls) > 1:
                            muls.pop(0)()
                    pend = (p, es)
                muls.append(emit_attnv(c, pend[0], pend[1], vt, oT))
                tail_muls = muls
                prev = (c, oT)
            for m in tail_muls:
                m()
            emit_yproj(*prev)

    nc.finalize()
    return nc


def _get_nc(debug_dump=False):
    key = ("nc", debug_dump)
    if key not in _CACHE:
        _CACHE[key] = _build_nc(debug_dump)
    return _CACHE[key]


def _make_in_maps(x, q_w, q_b, k_w, k_b, v_w, v_b, o_w, o_b):
    x = np.asarray(x, np.float32)
    shared = {
        "wq": np.ascontiguousarray((np.asarray(q_w, np.float32).T * SCALE).astype(NPBF)),
        "wk": np.ascontiguousarray(np.asarray(k_w, np.float32).T.astype(NPBF)),
        "wv": np.ascontiguousarray(np.asarray(v_w, np.float32).T.astype(NPBF)),
        "wo": np.ascontiguousarray(np.asarray(o_w, np.float32).T.astype(NPBF)),
        "qb": np.ascontiguousarray(
            (np.asarray(q_b, np.float32) * SCALE).reshape(DT, P).T),
        "kb": np.ascontiguousarray(np.asarray(k_b, np.float32).reshape(DT, P).T),
        "vb": np.asarray(v_b, np.float32).reshape(1, D).copy(),
        "ob": np.asarray(o_b, np.float32).reshape(1, D).copy(),
    }
    in_maps = []
    for c in range(N_CORES):
        b, h = divmod(c, 2)
        xs = x[b, h * TOK:(h + 1) * TOK, :]
        m = dict(shared)
        m["xt"] = np.ascontiguousarray(xs.T.astype(NPBF))
        m["xn"] = np.ascontiguousarray(xs.astype(NPBF))
        in_maps.append(m)
    return in_maps


def run(trace=False, trace_cores=None, debug_dump=False, **inputs):
    nc = _get_nc(debug_dump)
    in_maps = _make_in_maps(**inputs)
    res = bass_utils.run_bass_kernel_spmd(
        nc, in_maps, core_ids=list(range(N_CORES)), trace=trace,
        trace_cores=trace_cores)
    out = np.empty((B, S, D), np.float32)
    for c in range(N_CORES):
        b, h = divmod(c, 2)
        out[b, h * TOK:(h + 1) * TOK, :] = res.results[c]["y"]
    return out, res


def kernel(**inputs) -> np.ndarray:
    out, _ = run(trace=False, **inputs)
    return out


# revision 22
# speedup vs baseline: 1.2213x; 1.0300x over previous
"""Chunked local attention (landmark-augmented) for 8 Trainium2 NeuronCores.

Model (see reference): B=4, S=8192, D=768, H=12 heads of 64, chunk=512,
NL=32 landmark tokens = mean of 32 evenly spaced 256-token segments.
Every chunk attends over [32 landmarks ; its own 512 tokens].

Sharding: core c handles batch b=c//2, tokens [h*4096, (h+1)*4096), h=c%2.
Each core computes the 16 landmark partial means of its own half-sequence
and all-gathers the other 16 from its pair core ({2b, 2b+1}).

Layout strategy (matmuls in bf16, fp32 psum; weight loads hide via FWL):
  - host passes x shard both natural [4096,768] (landmark sums only) and
    transposed [768,4096] (everything else), plus W.T weights [din,dout],
    all pre-cast to bf16.
  - Q_T, K_T computed transposed [768, 512] per chunk; V natural [512, 768].
  - scores computed transposed [ctx, 512] so softmax-sum and attn@V both
    contract over ctx on the partition dim; Z row-sums via ones-matmuls
    packed into free column strips of the array.
  - attention out O_T [768, 512] feeds the output projection directly.
  - emission is software-pipelined: chunk c's output projection is emitted
    after chunk c+1's QKV so the in-order PE queue never stalls on the
    softmax-normalize DMA round trip.
"""

import sys

for _p in ("/opt/trn_rl_repo", "/root/.axon_site/_ro/trn_rl_repo"):
    if _p not in sys.path:
        sys.path.append(_p)

import numpy as np
import ml_dtypes

import concourse.bacc as bacc
import concourse.tile as tile
from concourse import mybir
from concourse import bass_utils

B, S, D = 4, 8192, 768
H, HD = 12, 64
CS, NL = 512, 32
N_CORES = 8
TOK = B * S // N_CORES      # 4096 tokens per core
NCH = TOK // CS             # 8 chunks per core
SEG = S // NL               # 256 tokens per landmark segment
NSEG_LOC = TOK // SEG       # 16 local segments per core
DT = D // 128               # 6 din/dout tiles
P = 128
SCALE = HD ** -0.5          # 0.125

F32 = mybir.dt.float32
BF16 = mybir.dt.bfloat16
NPBF = ml_dtypes.bfloat16
EXP = mybir.ActivationFunctionType.Exp
IDENT = mybir.ActivationFunctionType.Identity

_CACHE = {}


def _build_nc(debug_dump=False):
    nc = bacc.Bacc("TRN2", target_bir_lowering=False, debug=False,
                   num_devices=N_CORES)

    # ---- DRAM I/O --------------------------------------------------------
    xt_d = nc.dram_tensor("xt", [D, TOK], BF16, kind="ExternalInput")
    xn_d = nc.dram_tensor("xn", [TOK, D], BF16, kind="ExternalInput")
    w_d = {k: nc.dram_tensor(f"w{k}", [D, D], BF16, kind="ExternalInput")
           for k in "qkvo"}
    qb_d = nc.dram_tensor("qb", [P, DT], F32, kind="ExternalInput")
    kb_d = nc.dram_tensor("kb", [P, DT], F32, kind="ExternalInput")
    vb_d = nc.dram_tensor("vb", [1, D], F32, kind="ExternalInput")
    ob_d = nc.dram_tensor("ob", [1, D], F32, kind="ExternalInput")
    y_d = nc.dram_tensor("y", [TOK, D], F32, kind="ExternalOutput")

    dbg = {}

    with tile.TileContext(nc) as tc:
        with (
            tc.tile_pool(name="wpool", bufs=1) as wpool,
            tc.tile_pool(name="singles", bufs=1) as singles,
            tc.tile_pool(name="sb", bufs=2) as sb,
            tc.tile_pool(name="psa", bufs=2, space="PSUM") as psa,
            tc.tile_pool(name="pss", bufs=3, space="PSUM") as pss,
            tc.tile_pool(name="pso", bufs=2, space="PSUM") as pso_pool,
            tc.tile_pool(name="psz", bufs=1, space="PSUM") as psz_pool,
            tc.tile_pool(name="dram", bufs=1, space="DRAM") as dram,
        ):
            if debug_dump:
                for nm, shp in [("lm_loc", [NSEG_LOC, D]), ("lm_nat", [NL, D]),
                                ("lm_T0", [P, NL]), ("klm_T0", [P, NL]),
                                ("vlm_d", [NL, D]), ("qT00", [P, CS]),
                                ("kT00", [P, CS]), ("v00", [P, D]),
                                ("e0_p0k0", [NL, CS]), ("e0_p0k1", [P, CS]),
                                ("e1_p0k1", [P, CS]), ("z_p0", [P, CS]),
                                ("rzb_p0", [P, CS]), ("oT00", [P, CS])]:
                    dbg[nm] = nc.dram_tensor("dbg_" + nm, shp, F32,
                                             kind="ExternalOutput")

            def dump(nm, ap):
                if nm in dbg:
                    t = sb.tile(list(ap.shape), F32, name="dmp_" + nm,
                                tag="dmp", bufs=2)
                    nc.vector.tensor_copy(t[:], ap)
                    nc.sync.dma_start(
                        dbg[nm][tuple(slice(0, s) for s in ap.shape)], t[:])

            # ---- constants / weights ------------------------------------
            ones = singles.tile([P, 1], BF16, name="ones")
            nc.vector.memset(ones[:], 1.0)
            # indicator patterns for landmark sums: ind[:, 16*s + s] = 1
            ind = singles.tile([P, 16 * NSEG_LOC], BF16, name="ind")
            nc.vector.memset(ind[:], 0.0)
            for s in range(NSEG_LOC):
                nc.vector.memset(ind[:, 16 * s + s: 16 * s + s + 1], 1.0)
            ident32 = singles.tile([32, 32], F32, name="ident32")
            from concourse.masks import make_identity
            make_identity(nc, ident32[:])

            wsb = {}
            for k in "vqko":
                for d in range(DT):
                    t = wpool.tile([P, D], BF16, name=f"w{k}{d}", tag=f"w{k}{d}")
                    nc.scalar.dma_start(t[:], w_d[k][d * P:(d + 1) * P, :])
                    wsb[k, d] = t
            qb_sb = singles.tile([P, DT], F32, name="qb_sb")
            kb_sb = singles.tile([P, DT], F32, name="kb_sb")
            nc.scalar.dma_start(qb_sb[:], qb_d[:, :])
            nc.scalar.dma_start(kb_sb[:], kb_d[:, :])
            vb_bc = singles.tile([P, D], F32, name="vb_bc")
            ob_bc = singles.tile([P, D], F32, name="ob_bc")
            nc.scalar.dma_start(vb_bc[:], vb_d[0:1, :].partition_broadcast(P))
            nc.scalar.dma_start(ob_bc[:], ob_d[0:1, :].partition_broadcast(P))

            # ---- landmark partial sums (natural layout, via indicators) --
            # landmark accumulators borrow the attention-output psum slots:
            # those are first needed by chunk-0 attention, which already
            # depends on the landmarks, so this adds no serialization.
            lm_ps = pso_pool.tile([NSEG_LOC, 512], F32, name="lm_ps", tag="pso")
            lm_ps2 = psz_pool.tile([NSEG_LOC, 256], F32, name="lm_ps2", tag="psz")
            for tt in range(TOK // P):  # 32 token tiles
                xn_t = sb.tile([P, D], BF16, name=f"xn{tt}", tag="v", bufs=6)
                nc.gpsimd.dma_start(xn_t[:], xn_d[tt * P:(tt + 1) * P, :])
                s = tt * P // SEG
                lhs = ind[:, 16 * s: 16 * (s + 1)]
                nc.tensor.matmul(lm_ps[:, :], lhs, xn_t[:, 0:512],
                                 start=(tt == 0), stop=(tt == TOK // P - 1))
                nc.tensor.matmul(lm_ps2[:, :], lhs, xn_t[:, 512:D],
                                 start=(tt == 0), stop=(tt == TOK // P - 1))

            lm_loc = sb.tile([NSEG_LOC, D], F32, name="lm_loc", tag="rz", bufs=3)
            nc.scalar.copy(lm_loc[:, 0:512], lm_ps[:, :])
            nc.scalar.copy(lm_loc[:, 512:D], lm_ps2[:, :])
            dump("lm_loc", lm_loc[:, :])
            lm_in_b = dram.tile([NSEG_LOC, D], F32, name="lm_in_b")
            lm_out_b = dram.tile([NL, D], F32, name="lm_out_b")
            nc.gpsimd.dma_start(lm_in_b[:, :], lm_loc[:, :])
            nc.gpsimd.collective_compute(
                "AllGather", mybir.AluOpType.bypass,
                replica_groups=[[0, 1], [2, 3], [4, 5], [6, 7]],
                ins=[lm_in_b.opt()], outs=[lm_out_b.opt()],
            )
            lm_nat = sb.tile([NL, D], F32, name="lm_nat", tag="rz", bufs=3)
            nc.gpsimd.dma_start(lm_nat[:], lm_out_b[:, :])
            dump("lm_nat", lm_nat[:, :])

            # transpose to lm_T [din, 32] per din tile, scaled by 1/SEG
            lm_T = []
            for d in range(DT):
                ps_t = pss.tile([P, NL], F32, name=f"lmT_ps{d}", tag="pss")
                nc.tensor.transpose(ps_t[:], lm_nat[:, d * P:(d + 1) * P],
                                    ident32[:])
                t = singles.tile([P, NL], BF16, name=f"lmT{d}")
                nc.scalar.activation(t[:], ps_t[:], IDENT, scale=1.0 / SEG)
                if d == 0:
                    dump("lm_T0", t[:])
                lm_T.append(t)

            # K_lm_T [dout, 32] per dout tile (+ k bias); V_lm [32, 768] (+ v bias)
            klm_T = []
            for m in range(DT):
                ps_k = pss.tile([P, NL], F32, name=f"klm_ps{m}", tag="pss")
                for d in range(DT):
                    nc.tensor.matmul(ps_k[:], wsb["k", d][:, m * P:(m + 1) * P],
                                     lm_T[d][:], start=(d == 0), stop=(d == DT - 1))
                t = singles.tile([P, NL], BF16, name=f"klmT{m}")
                nc.scalar.activation(t[:], ps_k[:], IDENT, bias=kb_sb[:, m:m + 1])
                if m == 0:
                    dump("klm_T0", t[:])
                klm_T.append(t)
            ps_v1 = pss.tile([NL, 512], F32, name="ps_vlm1", tag="pss")
            ps_v2 = pss.tile([NL, 256], F32, name="ps_vlm2", tag="pss")
            for d in range(DT):
                nc.tensor.matmul(ps_v1[:], lm_T[d][:], wsb["v", d][:, 0:512],
                                 start=(d == 0), stop=(d == DT - 1))
                nc.tensor.matmul(ps_v2[:], lm_T[d][:], wsb["v", d][:, 512:D],
                                 start=(d == 0), stop=(d == DT - 1))
            vlm = singles.tile([NL, D], BF16, name="vlm")
            nc.vector.tensor_add(vlm[:, 0:512], ps_v1[:], vb_bc[0:NL, 0:512])
            nc.vector.tensor_add(vlm[:, 512:D], ps_v2[:], vb_bc[0:NL, 512:D])
            dump("vlm_d", vlm[:, :])

            # ---- main chunk pipeline (emission software-pipelined) -------
            def emit_proj(c):
                """QKV projections for chunk c; returns (qT, kT, vt)."""
                T0 = c * CS
                xt_c = []
                for d in range(DT):
                    t = sb.tile([P, CS], BF16, name=f"xt{c}_{d}", tag="xt",
                                bufs=14)
                    nc.sync.dma_start(t[:], xt_d[d * P:(d + 1) * P, T0:T0 + CS])
                    xt_c.append(t)
                # V first: its psum slots are evicted by the (slower) DVE, and
                # doing it first lets those evictions overlap the Q/K matmuls,
                # so later psum reuse never waits on DVE.
                vt = []
                for tt in range(CS // P):
                    ps_1 = psa.tile([P, 512], F32, name=f"v1_ps{c}_{tt}", tag="psa")
                    ps_2 = psa.tile([P, 256], F32, name=f"v2_ps{c}_{tt}", tag="psa")
                    for d in range(DT):
                        nc.tensor.matmul(ps_1[:], xt_c[d][:, tt * P:(tt + 1) * P],
                                         wsb["v", d][:, 0:512], start=(d == 0),
                                         stop=(d == DT - 1))
                        nc.tensor.matmul(ps_2[:], xt_c[d][:, tt * P:(tt + 1) * P],
                                         wsb["v", d][:, 512:D], start=(d == 0),
                                         stop=(d == DT - 1))
                    t = sb.tile([P, D], BF16, name=f"v{c}_{tt}", tag="v", bufs=6)
                    nc.vector.tensor_add(t[:, 0:512], ps_1[:], vb_bc[:, 0:512])
                    nc.vector.tensor_add(t[:, 512:D], ps_2[:], vb_bc[:, 512:D])
                    if c == 0 and tt == 0:
                        dump("v00", t[:])
                    vt.append(t)
                qT, kT = [], []
                for m in range(DT):
                    ps_q = psa.tile([P, CS], F32, name=f"q_ps{c}_{m}", tag="psa")
                    for d in range(DT):
                        nc.tensor.matmul(ps_q[:], wsb["q", d][:, m * P:(m + 1) * P],
                                         xt_c[d][:], start=(d == 0),
                                         stop=(d == DT - 1))
                    t = sb.tile([P, CS], BF16, name=f"qT{c}_{m}", tag="qT", bufs=8)
                    nc.vector.tensor_scalar_add(t[:], ps_q[:], qb_sb[:, m:m + 1])
                    if c == 0 and m == 0:
                        dump("qT00", t[:])
                    qT.append(t)
                for m in range(DT):
                    ps_k = psa.tile([P, CS], F32, name=f"k_ps{c}_{m}", tag="psa")
                    for d in range(DT):
                        nc.tensor.matmul(ps_k[:], wsb["k", d][:, m * P:(m + 1) * P],
                                         xt_c[d][:], start=(d == 0),
                                         stop=(d == DT - 1))
                    t = sb.tile([P, CS], BF16, name=f"kT{c}_{m}", tag="kT", bufs=8)
                    nc.vector.tensor_scalar_add(t[:], ps_k[:], kb_sb[:, m:m + 1])
                    if c == 0 and m == 0:
                        dump("kT00", t[:])
                    kT.append(t)
                return qT, kT, vt

            def emit_scores(c, p, qT, kT):
                """Scores + exp for head pair p; returns the 10 e tiles."""
                es = []
                for kt in range(5):
                    if kt == 0:
                        ksz = NL
                        k0 = klm_T[p][0:64, :]
                        k1 = klm_T[p][64:P, :]
                    else:
                        j = kt - 1
                        ksz = P
                        k0 = kT[p][0:64, j * P:(j + 1) * P]
                        k1 = kT[p][64:P, j * P:(j + 1) * P]
                    s0 = pss.tile([P, CS], F32, name=f"s0_{c}_{p}_{kt}",
                                  tag="pss")
                    s1 = pss.tile([P, CS], F32, name=f"s1_{c}_{p}_{kt}",
                                  tag="pss")
                    nc.tensor.matmul(s0[0:ksz, :], k0, qT[p][0:64, :],
                                     start=True, stop=True)
                    nc.tensor.matmul(s1[0:ksz, :], k1, qT[p][64:P, :],
                                     start=True, stop=True)
                    e0 = sb.tile([P, CS], BF16, name=f"e0_{c}_{p}_{kt}",
                                 tag="e", bufs=22)
                    e1 = sb.tile([P, CS], BF16, name=f"e1_{c}_{p}_{kt}",
                                 tag="e", bufs=22)
                    nc.scalar.activation(e0[0:ksz, :], s0[0:ksz, :], EXP)
                    nc.scalar.activation(e1[0:ksz, :], s1[0:ksz, :], EXP)
                    if c == 0 and p == 0 and kt == 0:
                        dump("e0_p0k0", e0[0:ksz, :])
                    if c == 0 and p == 0 and kt == 1:
                        dump("e0_p0k1", e0[0:ksz, :])
                        dump("e1_p0k1", e1[0:ksz, :])
                    es.append((e0, e1, ksz))
                return es

            def emit_attnv(c, p, es, vt, oT):
                """attn@V + softmax denominator + normalize for pair p."""
                ps_o = pso_pool.tile([P, CS], F32, name=f"o_ps{c}_{p}",
                                     tag="pso")
                ps_z = psz_pool.tile([P, CS], F32, name=f"z_ps{c}_{p}",
                                     tag="psz")
                for kt, (e0, e1, ksz) in enumerate(es):
                    if kt == 0:
                        v0 = vlm[0:NL, p * P: p * P + 64]
                        v1 = vlm[0:NL, p * P + 64: (p + 1) * P]
                    else:
                        j = kt - 1
                        v0 = vt[j][:, p * P: p * P + 64]
                        v1 = vt[j][:, p * P + 64: (p + 1) * P]
                    nc.tensor.matmul(ps_o[0:64, :], v0[0:ksz, :], e0[0:ksz, :],
                                     start=(kt == 0), stop=(kt == 4),
                                     tile_position=(0, 0))
                    nc.tensor.matmul(ps_o[64:P, :], v1[0:ksz, :], e1[0:ksz, :],
                                     start=(kt == 0), stop=(kt == 4),
                                     tile_position=(0, 64))
                    nc.tensor.matmul(ps_z[0:1, :], ones[0:ksz, :], e0[0:ksz, :],
                                     start=(kt == 0), stop=(kt == 4),
                                     tile_position=(0, 0))
                    nc.tensor.matmul(ps_z[32:33, :], ones[0:ksz, :],
                                     e1[0:ksz, :], start=(kt == 0),
                                     stop=(kt == 4), tile_position=(0, 32))
                if c == 0 and p == 0:
                    dump("z_p0", ps_z[:, :])
                # free the Z bank first (recips), then the O bank (copy); the
                # normalize itself is deferred so the denominator's DMA
                # broadcast round-trip never blocks the DVE queue.
                rz = sb.tile([33, CS], F32, name=f"rz{c}_{p}", tag="rz",
                             bufs=3)
                nc.vector.reciprocal(rz[0:1, :], ps_z[0:1, :])
                nc.vector.reciprocal(rz[32:33, :], ps_z[32:33, :])
                oTu = sb.tile([P, CS], F32, name=f"oTu{c}_{p}", tag="oTu",
                              bufs=4)
                nc.vector.tensor_copy(oTu[:], ps_o[:])
                zdram = dram.tile([2, CS], F32, name=f"zd{c}_{p}", tag="zd",
                                  bufs=4)
                nc.gpsimd.dma_start(zdram[0:1, :], rz[0:1, :])
                nc.gpsimd.dma_start(zdram[1:2, :], rz[32:33, :])
                rzb = sb.tile([P, CS], F32, name=f"rzb{c}_{p}", tag="rzb",
                              bufs=3)
                nc.gpsimd.dma_start(rzb[0:64, :],
                                    zdram[0:1, :].partition_broadcast(64))
                nc.gpsimd.dma_start(rzb[64:P, :],
                                    zdram[1:2, :].partition_broadcast(64))
                if c == 0 and p == 0:
                    dump("rzb_p0", rzb[:, :])
                t = sb.tile([P, CS], BF16, name=f"oT{c}_{p}", tag="oT",
                            bufs=8)
                oT.append(t)

                def mul(t=t, oTu=oTu, rzb=rzb, first=(c == 0 and p == 0)):
                    nc.vector.tensor_mul(t[:], oTu[:], rzb[:])
                    if first:
                        dump("oT00", t[:])
                return mul

            def emit_yproj(c, oT):
                T0 = c * CS
                for tt in range(CS // P):
                    ps_1 = psa.tile([P, 512], F32, name=f"y1_ps{c}_{tt}", tag="psa")
                    ps_2 = psa.tile([P, 256], F32, name=f"y2_ps{c}_{tt}", tag="psa")
                    for d in range(DT):
                        nc.tensor.matmul(ps_1[:], oT[d][:, tt * P:(tt + 1) * P],
                                         wsb["o", d][:, 0:512], start=(d == 0),
                                         stop=(d == DT - 1))
                        nc.tensor.matmul(ps_2[:], oT[d][:, tt * P:(tt + 1) * P],
                                         wsb["o", d][:, 512:D], start=(d == 0),
                                         stop=(d == DT - 1))
                    t = sb.tile([P, D], F32, name=f"y{c}_{tt}", tag="y", bufs=3)
                    nc.vector.tensor_add(t[:, 0:512], ps_1[:], ob_bc[:, 0:512])
                    nc.vector.tensor_add(t[:, 512:D], ps_2[:], ob_bc[:, 512:D])
                    nc.sync.dma_start(
                        y_d[T0 + tt * P: T0 + (tt + 1) * P, :], t[:])

            # software-pipelined emission: within a chunk, the scores of pair
            # p are emitted before attn@V of pair p-1, so the PE always has a
            # dense stream of ready work while the exps for the younger pair
            # run on the scalar engine. Output projection lags one chunk.
            prev = None  # (c, oT) awaiting output projection
            tail_muls = []
            for c in range(NCH):
                qT, kT, vt = emit_proj(c)
                for m in tail_muls:
                    m()
                tail_muls = []
                if prev is not None:
                    emit_yproj(*prev)
                oT = []
                pend = None   # pair whose scores are done, attnV pending
                muls = []     # deferred normalizes, flushed one pair late
                for p in range(DT):
                    es = emit_scores(c, p, qT, kT)
                    if pend is not None:
                        muls.append(emit_attnv(c, pend[0], pend[1], vt, oT))
                        if len(muls) > 1:
                            muls.pop(0)()
                    pend = (p, es)
                muls.append(emit_attnv(c, pend[0], pend[1], vt, oT))
                tail_muls = muls
                prev = (c, oT)
            for m in tail_muls:
                m()
            emit_yproj(*prev)

    nc.finalize()
    return nc


def _get_nc(debug_dump=False):
    key = ("nc", debug_dump)
    if key not in _CACHE:
        _CACHE[key] = _build_nc(debug_dump)
    return _CACHE[key]


def _make_in_maps(x, q_w, q_b, k_w, k_b, v_w, v_b, o_w, o_b):
    x = np.asarray(x, np.float32)
    shared = {
        "wq": np.ascontiguousarray((np.asarray(q_w, np.float32).T * SCALE).astype(NPBF)),
        "wk": np.ascontiguousarray(np.asarray(k_w, np.float32).T.astype(NPBF)),
        "wv": np.ascontiguousarray(np.asarray(v_w, np.float32).T.astype(NPBF)),
        "wo": np.ascontiguousarray(np.asarray(o_w, np.float32).T.astype(NPBF)),
        "qb": np.ascontiguousarray(
            (np.asarray(q_b, np.float32) * SCALE).reshape(DT, P).T),
        "kb": np.ascontiguousarray(np.asarray(k_b, np.float32).reshape(DT, P).T),
        "vb": np.asarray(v_b, np.float32).reshape(1, D).copy(),
        "ob": np.asarray(o_b, np.float32).reshape(1, D).copy(),
    }
    in_maps = []
    for c in range(N_CORES):
        b, h = divmod(c, 2)
        xs = x[b, h * TOK:(h + 1) * TOK, :]
        m = dict(shared)
        m["xt"] = np.ascontiguousarray(xs.T.astype(NPBF))
        m["xn"] = np.ascontiguousarray(xs.astype(NPBF))
        in_maps.append(m)
    return in_maps


def run(trace=False, trace_cores=None, debug_dump=False, **inputs):
    nc = _get_nc(debug_dump)
    in_maps = _make_in_maps(**inputs)
    res = bass_utils.run_bass_kernel_spmd(
        nc, in_maps, core_ids=list(range(N_CORES)), trace=trace,
        trace_cores=trace_cores)
    out = np.empty((B, S, D), np.float32)
    for c in range(N_CORES):
        b, h = divmod(c, 2)
        out[b, h * TOK:(h + 1) * TOK, :] = res.results[c]["y"]
    return out, res


def kernel(**inputs) -> np.ndarray:
    out, _ = run(trace=False, **inputs)
    return out


# revision 23
# speedup vs baseline: 1.2280x; 1.0055x over previous
"""Chunked local attention (landmark-augmented) for 8 Trainium2 NeuronCores.

Model (see reference): B=4, S=8192, D=768, H=12 heads of 64, chunk=512,
NL=32 landmark tokens = mean of 32 evenly spaced 256-token segments.
Every chunk attends over [32 landmarks ; its own 512 tokens].

Sharding: core c handles batch b=c//2, tokens [h*4096, (h+1)*4096), h=c%2.
Each core computes the 16 landmark partial means of its own half-sequence
and all-gathers the other 16 from its pair core ({2b, 2b+1}).

Layout strategy (matmuls in bf16, fp32 psum; weight loads hide via FWL):
  - host passes x shard both natural [4096,768] (landmark sums only) and
    transposed [768,4096] (everything else), plus W.T weights [din,dout],
    all pre-cast to bf16.
  - Q_T, K_T computed transposed [768, 512] per chunk; V natural [512, 768].
  - scores computed transposed [ctx, 512] so softmax-sum and attn@V both
    contract over ctx on the partition dim; Z row-sums via ones-matmuls
    packed into free column strips of the array.
  - attention out O_T [768, 512] feeds the output projection directly.
  - emission is software-pipelined: chunk c's output projection is emitted
    after chunk c+1's QKV so the in-order PE queue never stalls on the
    softmax-normalize DMA round trip.
"""

import sys

for _p in ("/opt/trn_rl_repo", "/root/.axon_site/_ro/trn_rl_repo"):
    if _p not in sys.path:
        sys.path.append(_p)

import numpy as np
import ml_dtypes

import concourse.bacc as bacc
import concourse.tile as tile
from concourse import mybir
from concourse import bass_utils

B, S, D = 4, 8192, 768
H, HD = 12, 64
CS, NL = 512, 32
N_CORES = 8
TOK = B * S // N_CORES      # 4096 tokens per core
NCH = TOK // CS             # 8 chunks per core
SEG = S // NL               # 256 tokens per landmark segment
NSEG_LOC = TOK // SEG       # 16 local segments per core
DT = D // 128               # 6 din/dout tiles
P = 128
SCALE = HD ** -0.5          # 0.125

F32 = mybir.dt.float32
BF16 = mybir.dt.bfloat16
NPBF = ml_dtypes.bfloat16
EXP = mybir.ActivationFunctionType.Exp
IDENT = mybir.ActivationFunctionType.Identity

_CACHE = {}


def _build_nc(debug_dump=False):
    nc = bacc.Bacc("TRN2", target_bir_lowering=False, debug=False,
                   num_devices=N_CORES)

    # ---- DRAM I/O --------------------------------------------------------
    xt_d = nc.dram_tensor("xt", [D, TOK], BF16, kind="ExternalInput")
    xn_d = nc.dram_tensor("xn", [TOK, D], BF16, kind="ExternalInput")
    w_d = {k: nc.dram_tensor(f"w{k}", [D, D], BF16, kind="ExternalInput")
           for k in "qkvo"}
    qb_d = nc.dram_tensor("qb", [P, DT], F32, kind="ExternalInput")
    kb_d = nc.dram_tensor("kb", [P, DT], F32, kind="ExternalInput")
    vb_d = nc.dram_tensor("vb", [1, D], F32, kind="ExternalInput")
    ob_d = nc.dram_tensor("ob", [1, D], F32, kind="ExternalInput")
    y_d = nc.dram_tensor("y", [TOK, D], F32, kind="ExternalOutput")

    dbg = {}

    with tile.TileContext(nc) as tc:
        with (
            tc.tile_pool(name="wpool", bufs=1) as wpool,
            tc.tile_pool(name="singles", bufs=1) as singles,
            tc.tile_pool(name="sb", bufs=2) as sb,
            tc.tile_pool(name="psa", bufs=2, space="PSUM") as psa,
            tc.tile_pool(name="pss", bufs=3, space="PSUM") as pss,
            tc.tile_pool(name="pso", bufs=2, space="PSUM") as pso_pool,
            tc.tile_pool(name="psz", bufs=1, space="PSUM") as psz_pool,
            tc.tile_pool(name="dram", bufs=1, space="DRAM") as dram,
        ):
            if debug_dump:
                for nm, shp in [("lm_loc", [NSEG_LOC, D]), ("lm_nat", [NL, D]),
                                ("lm_T0", [P, NL]), ("klm_T0", [P, NL]),
                                ("vlm_d", [NL, D]), ("qT00", [P, CS]),
                                ("kT00", [P, CS]), ("v00", [P, D]),
                                ("e0_p0k0", [NL, CS]), ("e0_p0k1", [P, CS]),
                                ("e1_p0k1", [P, CS]), ("z_p0", [P, CS]),
                                ("rzb_p0", [P, CS]), ("oT00", [P, CS])]:
                    dbg[nm] = nc.dram_tensor("dbg_" + nm, shp, F32,
                                             kind="ExternalOutput")

            def dump(nm, ap):
                if nm in dbg:
                    t = sb.tile(list(ap.shape), F32, name="dmp_" + nm,
                                tag="dmp", bufs=2)
                    nc.vector.tensor_copy(t[:], ap)
                    nc.sync.dma_start(
                        dbg[nm][tuple(slice(0, s) for s in ap.shape)], t[:])

            # ---- constants / weights ------------------------------------
            ones = singles.tile([P, 1], BF16, name="ones")
            nc.vector.memset(ones[:], 1.0)
            # indicator patterns for landmark sums: ind[:, 16*s + s] = 1
            ind = singles.tile([P, 16 * NSEG_LOC], BF16, name="ind")
            nc.vector.memset(ind[:], 0.0)
            for s in range(NSEG_LOC):
                nc.vector.memset(ind[:, 16 * s + s: 16 * s + s + 1], 1.0)
            ident32 = singles.tile([32, 32], F32, name="ident32")
            from concourse.masks import make_identity
            make_identity(nc, ident32[:])

            wsb = {}
            for k in "vqko":
                for d in range(DT):
                    t = wpool.tile([P, D], BF16, name=f"w{k}{d}", tag=f"w{k}{d}")
                    nc.scalar.dma_start(t[:], w_d[k][d * P:(d + 1) * P, :])
                    wsb[k, d] = t
            qb_sb = singles.tile([P, DT], F32, name="qb_sb")
            kb_sb = singles.tile([P, DT], F32, name="kb_sb")
            nc.scalar.dma_start(qb_sb[:], qb_d[:, :])
            nc.scalar.dma_start(kb_sb[:], kb_d[:, :])
            vb_bc = singles.tile([P, D], F32, name="vb_bc")
            ob_bc = singles.tile([P, D], F32, name="ob_bc")
            nc.scalar.dma_start(vb_bc[:], vb_d[0:1, :].partition_broadcast(P))
            nc.scalar.dma_start(ob_bc[:], ob_d[0:1, :].partition_broadcast(P))

            # ---- landmark partial sums (natural layout, via indicators) --
            # landmark accumulators borrow the attention-output psum slots:
            # those are first needed by chunk-0 attention, which already
            # depends on the landmarks, so this adds no serialization.
            lm_ps = pso_pool.tile([NSEG_LOC, 512], F32, name="lm_ps", tag="pso")
            lm_ps2 = psz_pool.tile([NSEG_LOC, 256], F32, name="lm_ps2", tag="psz")
            for tt in range(TOK // P):  # 32 token tiles
                xn_t = sb.tile([P, D], BF16, name=f"xn{tt}", tag="v", bufs=6)
                nc.gpsimd.dma_start(xn_t[:], xn_d[tt * P:(tt + 1) * P, :])
                s = tt * P // SEG
                lhs = ind[:, 16 * s: 16 * (s + 1)]
                nc.tensor.matmul(lm_ps[:, :], lhs, xn_t[:, 0:512],
                                 start=(tt == 0), stop=(tt == TOK // P - 1))
                nc.tensor.matmul(lm_ps2[:, :], lhs, xn_t[:, 512:D],
                                 start=(tt == 0), stop=(tt == TOK // P - 1))

            lm_loc = sb.tile([NSEG_LOC, D], F32, name="lm_loc", tag="rz", bufs=3)
            nc.scalar.copy(lm_loc[:, 0:512], lm_ps[:, :])
            nc.scalar.copy(lm_loc[:, 512:D], lm_ps2[:, :])
            dump("lm_loc", lm_loc[:, :])
            lm_in_b = dram.tile([NSEG_LOC, D], F32, name="lm_in_b")
            lm_out_b = dram.tile([NL, D], F32, name="lm_out_b")
            nc.gpsimd.dma_start(lm_in_b[:, :], lm_loc[:, :])
            nc.gpsimd.collective_compute(
                "AllGather", mybir.AluOpType.bypass,
                replica_groups=[[0, 1], [2, 3], [4, 5], [6, 7]],
                ins=[lm_in_b.opt()], outs=[lm_out_b.opt()],
            )
            lm_nat = sb.tile([NL, D], F32, name="lm_nat", tag="rz", bufs=3)
            nc.gpsimd.dma_start(lm_nat[:], lm_out_b[:, :])
            dump("lm_nat", lm_nat[:, :])

            # transpose to lm_T [din, 32] per din tile, scaled by 1/SEG
            lm_T = []
            for d in range(DT):
                ps_t = pss.tile([P, NL], F32, name=f"lmT_ps{d}", tag="pss")
                nc.tensor.transpose(ps_t[:], lm_nat[:, d * P:(d + 1) * P],
                                    ident32[:])
                t = singles.tile([P, NL], BF16, name=f"lmT{d}")
                nc.scalar.activation(t[:], ps_t[:], IDENT, scale=1.0 / SEG)
                if d == 0:
                    dump("lm_T0", t[:])
                lm_T.append(t)

            # K_lm_T [dout, 32] per dout tile (+ k bias); V_lm [32, 768] (+ v bias)
            klm_T = []
            for m in range(DT):
                ps_k = pss.tile([P, NL], F32, name=f"klm_ps{m}", tag="pss")
                for d in range(DT):
                    nc.tensor.matmul(ps_k[:], wsb["k", d][:, m * P:(m + 1) * P],
                                     lm_T[d][:], start=(d == 0), stop=(d == DT - 1))
                t = singles.tile([P, NL], BF16, name=f"klmT{m}")
                nc.scalar.activation(t[:], ps_k[:], IDENT, bias=kb_sb[:, m:m + 1])
                if m == 0:
                    dump("klm_T0", t[:])
                klm_T.append(t)
            ps_v1 = pss.tile([NL, 512], F32, name="ps_vlm1", tag="pss")
            ps_v2 = pss.tile([NL, 256], F32, name="ps_vlm2", tag="pss")
            for d in range(DT):
                nc.tensor.matmul(ps_v1[:], lm_T[d][:], wsb["v", d][:, 0:512],
                                 start=(d == 0), stop=(d == DT - 1))
                nc.tensor.matmul(ps_v2[:], lm_T[d][:], wsb["v", d][:, 512:D],
                                 start=(d == 0), stop=(d == DT - 1))
            vlm = singles.tile([NL, D], BF16, name="vlm")
            nc.vector.tensor_add(vlm[:, 0:512], ps_v1[:], vb_bc[0:NL, 0:512])
            nc.vector.tensor_add(vlm[:, 512:D], ps_v2[:], vb_bc[0:NL, 512:D])
            dump("vlm_d", vlm[:, :])

            # ---- main chunk pipeline (emission software-pipelined) -------
            def emit_proj(c):
                """QKV projections for chunk c; returns (qT, kT, vt)."""
                T0 = c * CS
                xt_c = []
                for d in range(DT):
                    t = sb.tile([P, CS], BF16, name=f"xt{c}_{d}", tag="xt",
                                bufs=14)
                    nc.sync.dma_start(t[:], xt_d[d * P:(d + 1) * P, T0:T0 + CS])
                    xt_c.append(t)
                # V first: its psum slots are evicted by the (slower) DVE, and
                # doing it first lets those evictions overlap the Q/K matmuls,
                # so later psum reuse never waits on DVE.
                vt = []
                for tt in range(CS // P):
                    ps_1 = psa.tile([P, 512], F32, name=f"v1_ps{c}_{tt}", tag="psa")
                    ps_2 = psa.tile([P, 256], F32, name=f"v2_ps{c}_{tt}", tag="psa")
                    for d in range(DT):
                        nc.tensor.matmul(ps_1[:], xt_c[d][:, tt * P:(tt + 1) * P],
                                         wsb["v", d][:, 0:512], start=(d == 0),
                                         stop=(d == DT - 1))
                        nc.tensor.matmul(ps_2[:], xt_c[d][:, tt * P:(tt + 1) * P],
                                         wsb["v", d][:, 512:D], start=(d == 0),
                                         stop=(d == DT - 1))
                    t = sb.tile([P, D], BF16, name=f"v{c}_{tt}", tag="v", bufs=6)
                    nc.vector.tensor_add(t[:, 0:512], ps_1[:], vb_bc[:, 0:512])
                    nc.vector.tensor_add(t[:, 512:D], ps_2[:], vb_bc[:, 512:D])
                    if c == 0 and tt == 0:
                        dump("v00", t[:])
                    vt.append(t)
                qT, kT = [], []
                for m in range(DT):
                    ps_q = psa.tile([P, CS], F32, name=f"q_ps{c}_{m}", tag="psa")
                    for d in range(DT):
                        nc.tensor.matmul(ps_q[:], wsb["q", d][:, m * P:(m + 1) * P],
                                         xt_c[d][:], start=(d == 0),
                                         stop=(d == DT - 1))
                    t = sb.tile([P, CS], BF16, name=f"qT{c}_{m}", tag="qT", bufs=8)
                    nc.vector.tensor_scalar_add(t[:], ps_q[:], qb_sb[:, m:m + 1])
                    if c == 0 and m == 0:
                        dump("qT00", t[:])
                    qT.append(t)
                for m in range(DT):
                    ps_k = psa.tile([P, CS], F32, name=f"k_ps{c}_{m}", tag="psa")
                    for d in range(DT):
                        nc.tensor.matmul(ps_k[:], wsb["k", d][:, m * P:(m + 1) * P],
                                         xt_c[d][:], start=(d == 0),
                                         stop=(d == DT - 1))
                    t = sb.tile([P, CS], BF16, name=f"kT{c}_{m}", tag="kT", bufs=8)
                    nc.vector.tensor_scalar_add(t[:], ps_k[:], kb_sb[:, m:m + 1])
                    if c == 0 and m == 0:
                        dump("kT00", t[:])
                    kT.append(t)
                return qT, kT, vt

            def emit_scores(c, p, qT, kT):
                """Scores + exp for head pair p; returns the 10 e tiles."""
                es = []
                for kt in range(5):
                    if kt == 0:
                        ksz = NL
                        k0 = klm_T[p][0:64, :]
                        k1 = klm_T[p][64:P, :]
                    else:
                        j = kt - 1
                        ksz = P
                        k0 = kT[p][0:64, j * P:(j + 1) * P]
                        k1 = kT[p][64:P, j * P:(j + 1) * P]
                    s0 = pss.tile([P, CS], F32, name=f"s0_{c}_{p}_{kt}",
                                  tag="pss")
                    s1 = pss.tile([P, CS], F32, name=f"s1_{c}_{p}_{kt}",
                                  tag="pss")
                    nc.tensor.matmul(s0[0:ksz, :], k0, qT[p][0:64, :],
                                     start=True, stop=True)
                    nc.tensor.matmul(s1[0:ksz, :], k1, qT[p][64:P, :],
                                     start=True, stop=True)
                    e0 = sb.tile([P, CS], BF16, name=f"e0_{c}_{p}_{kt}",
                                 tag="e", bufs=22)
                    e1 = sb.tile([P, CS], BF16, name=f"e1_{c}_{p}_{kt}",
                                 tag="e", bufs=22)
                    nc.scalar.activation(e0[0:ksz, :], s0[0:ksz, :], EXP)
                    nc.scalar.activation(e1[0:ksz, :], s1[0:ksz, :], EXP)
                    if c == 0 and p == 0 and kt == 0:
                        dump("e0_p0k0", e0[0:ksz, :])
                    if c == 0 and p == 0 and kt == 1:
                        dump("e0_p0k1", e0[0:ksz, :])
                        dump("e1_p0k1", e1[0:ksz, :])
                    es.append((e0, e1, ksz))
                return es

            def emit_attnv(c, p, es, vt, oT):
                """attn@V + softmax denominator + normalize for pair p."""
                ps_o = pso_pool.tile([P, CS], F32, name=f"o_ps{c}_{p}",
                                     tag="pso")
                ps_z = psz_pool.tile([P, CS], F32, name=f"z_ps{c}_{p}",
                                     tag="psz")
                for kt, (e0, e1, ksz) in enumerate(es):
                    if kt == 0:
                        v0 = vlm[0:NL, p * P: p * P + 64]
                        v1 = vlm[0:NL, p * P + 64: (p + 1) * P]
                    else:
                        j = kt - 1
                        v0 = vt[j][:, p * P: p * P + 64]
                        v1 = vt[j][:, p * P + 64: (p + 1) * P]
                    nc.tensor.matmul(ps_o[0:64, :], v0[0:ksz, :], e0[0:ksz, :],
                                     start=(kt == 0), stop=(kt == 4),
                                     tile_position=(0, 0))
                    nc.tensor.matmul(ps_o[64:P, :], v1[0:ksz, :], e1[0:ksz, :],
                                     start=(kt == 0), stop=(kt == 4),
                                     tile_position=(0, 64))
                    nc.tensor.matmul(ps_z[0:1, :], ones[0:ksz, :], e0[0:ksz, :],
                                     start=(kt == 0), stop=(kt == 4),
                                     tile_position=(0, 0))
                    nc.tensor.matmul(ps_z[32:33, :], ones[0:ksz, :],
                                     e1[0:ksz, :], start=(kt == 0),
                                     stop=(kt == 4), tile_position=(0, 32))
                if c == 0 and p == 0:
                    dump("z_p0", ps_z[:, :])
                # free the Z bank first (recips), then the O bank (copy); the
                # normalize itself is deferred so the denominator's DMA
                # broadcast round-trip never blocks the DVE queue.
                rz = sb.tile([33, CS], F32, name=f"rz{c}_{p}", tag="rz",
                             bufs=3)
                nc.vector.reciprocal(rz[0:1, :], ps_z[0:1, :])
                nc.vector.reciprocal(rz[32:33, :], ps_z[32:33, :])
                oTu = sb.tile([P, CS], F32, name=f"oTu{c}_{p}", tag="oTu",
                              bufs=4)
                nc.vector.tensor_copy(oTu[:], ps_o[:])
                zdram = dram.tile([2, CS], F32, name=f"zd{c}_{p}", tag="zd",
                                  bufs=4)
                nc.gpsimd.dma_start(zdram[0:1, :], rz[0:1, :])
                nc.gpsimd.dma_start(zdram[1:2, :], rz[32:33, :])
                rzb = sb.tile([P, CS], F32, name=f"rzb{c}_{p}", tag="rzb",
                              bufs=3)
                nc.gpsimd.dma_start(rzb[0:64, :],
                                    zdram[0:1, :].partition_broadcast(64))
                nc.gpsimd.dma_start(rzb[64:P, :],
                                    zdram[1:2, :].partition_broadcast(64))
                if c == 0 and p == 0:
                    dump("rzb_p0", rzb[:, :])
                t = sb.tile([P, CS], BF16, name=f"oT{c}_{p}", tag="oT",
                            bufs=8)
                # normalize runs on gpsimd: it queues behind the rzb DMAs on
                # the same engine, so no compute engine ever head-of-line
                # blocks on the denominator round-trip.
                nc.gpsimd.tensor_mul(t[:], oTu[:], rzb[:])
                if c == 0 and p == 0:
                    dump("oT00", t[:])
                oT.append(t)

            def emit_yproj(c, oT):
                T0 = c * CS
                for tt in range(CS // P):
                    ps_1 = psa.tile([P, 512], F32, name=f"y1_ps{c}_{tt}", tag="psa")
                    ps_2 = psa.tile([P, 256], F32, name=f"y2_ps{c}_{tt}", tag="psa")
                    for d in range(DT):
                        nc.tensor.matmul(ps_1[:], oT[d][:, tt * P:(tt + 1) * P],
                                         wsb["o", d][:, 0:512], start=(d == 0),
                                         stop=(d == DT - 1))
                        nc.tensor.matmul(ps_2[:], oT[d][:, tt * P:(tt + 1) * P],
                                         wsb["o", d][:, 512:D], start=(d == 0),
                                         stop=(d == DT - 1))
                    t = sb.tile([P, D], F32, name=f"y{c}_{tt}", tag="y", bufs=3)
                    nc.vector.tensor_add(t[:, 0:512], ps_1[:], ob_bc[:, 0:512])
                    nc.vector.tensor_add(t[:, 512:D], ps_2[:], ob_bc[:, 512:D])
                    nc.sync.dma_start(
                        y_d[T0 + tt * P: T0 + (tt + 1) * P, :], t[:])

            # software-pipelined emission: within a chunk, the scores of pair
            # p are emitted before attn@V of pair p-1, so the PE always has a
            # dense stream of ready work while the exps for the younger pair
            # run on the scalar engine. Output projection lags one chunk.
            prev = None  # (c, oT) awaiting output projection
            for c in range(NCH):
                qT, kT, vt = emit_proj(c)
                if prev is not None:
                    emit_yproj(*prev)
                oT = []
                pend = None   # pair whose scores are done, attnV pending
                for p in range(DT):
                    es = emit_scores(c, p, qT, kT)
                    if pend is not None:
                        emit_attnv(c, pend[0], pend[1], vt, oT)
                    pend = (p, es)
                emit_attnv(c, pend[0], pend[1], vt, oT)
                prev = (c, oT)
            emit_yproj(*prev)

    nc.finalize()
    return nc


def _get_nc(debug_dump=False):
    key = ("nc", debug_dump)
    if key not in _CACHE:
        _CACHE[key] = _build_nc(debug_dump)
    return _CACHE[key]


def _make_in_maps(x, q_w, q_b, k_w, k_b, v_w, v_b, o_w, o_b):
    x = np.asarray(x, np.float32)
    shared = {
        "wq": np.ascontiguousarray((np.asarray(q_w, np.float32).T * SCALE).astype(NPBF)),
        "wk": np.ascontiguousarray(np.asarray(k_w, np.float32).T.astype(NPBF)),
        "wv": np.ascontiguousarray(np.asarray(v_w, np.float32).T.astype(NPBF)),
        "wo": np.ascontiguousarray(np.asarray(o_w, np.float32).T.astype(NPBF)),
        "qb": np.ascontiguousarray(
            (np.asarray(q_b, np.float32) * SCALE).reshape(DT, P).T),
        "kb": np.ascontiguousarray(np.asarray(k_b, np.float32).reshape(DT, P).T),
        "vb": np.asarray(v_b, np.float32).reshape(1, D).copy(),
        "ob": np.asarray(o_b, np.float32).reshape(1, D).copy(),
    }
    in_maps = []
    for c in range(N_CORES):
        b, h = divmod(c, 2)
        xs = x[b, h * TOK:(h + 1) * TOK, :]
        m = dict(shared)
        m["xt"] = np.ascontiguousarray(xs.T.astype(NPBF))
        m["xn"] = np.ascontiguousarray(xs.astype(NPBF))
        in_maps.append(m)
    return in_maps


def run(trace=False, trace_cores=None, debug_dump=False, **inputs):
    nc = _get_nc(debug_dump)
    in_maps = _make_in_maps(**inputs)
    res = bass_utils.run_bass_kernel_spmd(
        nc, in_maps, core_ids=list(range(N_CORES)), trace=trace,
        trace_cores=trace_cores)
    out = np.empty((B, S, D), np.float32)
    for c in range(N_CORES):
        b, h = divmod(c, 2)
        out[b, h * TOK:(h + 1) * TOK, :] = res.results[c]["y"]
    return out, res


def kernel(**inputs) -> np.ndarray:
    out, _ = run(trace=False, **inputs)
    return out
